# revision 1
# baseline (speedup 1.0000x reference)
"""PoissonGaussianReadout forward on 8 trn2 NeuronCores.

Math (eval mode): each neuron n samples feat[b] (a [36,36,1024] image per
batch, 1024 = C*T channels) bilinearly at a fixed point mu[n], then takes a
per-neuron dot with W[n,:], adds b[n], applies elu(y)+1.

Strategy:
  - Batch-shard: 8 cores x 2 batches each; every core computes all 4096
    neurons for its 2 batches (min HBM traffic: 10.6MB x-shard + 16.8MB W).
  - Sort neurons by their bilinear base cell p00 = y0*36+x0.  A block of
    <=128 sorted neurons spans a contiguous window of flat positions
    [pfirst, pfirst+WIN).  One fp32 matmul per (block, d-chunk):
       psum[n, (b,j)] += Wblk[d, n]^T @ feat[b, pfirst+j, d-chunk]
    i.e. the moving operand is a *contiguous slice* of feat -- no gather.
  - Each neuron's 4 bilinear corners live at window offsets
    (p00-pfirst)+{0,1,36,37}; a host-built sparse mask [n, b, j] holds the
    bilinear weights there.  DVE: masked = psum * mask; reduce_j -> z[n, b].
  - Bias + elu(y)+1 = exp(-relu(-y)) + relu(y) on device, output assembled
    and un-permuted on host.

The block structure / masks depend on mu, which is known when kernel() is
called; the Bass program is traced fresh per call, so they are baked in as
compile-time constants (correct for any input values).
"""
import sys
sys.path.insert(0, "/opt/trn_rl_repo")

import numpy as np

from concourse import bass, mybir, tile
from concourse.bass_utils import run_bass_kernel_spmd
from concourse.vector_clock import ScopedClock
import bass_rust

# problem constants
B, C, T, HH, WW = 16, 64, 16, 36, 36
N, D = 4096, C * T            # 4096 neurons, 1024 input dim
P = HH * WW                   # 1296 flat positions
NCHUNK = 8                    # D / 128 contraction chunks
NCORES = 8
BPC = B // NCORES             # batches per core = 2
WINMAX = 256                  # psum bank: 2*WIN <= 512 fp32
PAD = 38                      # max corner offset (37) + 1
FEATW = P + PAD               # padded feat width per (chunk, batch)

F32 = mybir.dt.float32

# dtype for x / W (the matmul operands).  bf16 halves DMA traffic and
# roughly halves PE time (FWL weight loads + 1cyc/col stream); PSUM
# accumulation stays fp32.  Set to False for bit-accurate fp32.
USE_BF16 = True
if USE_BF16:
    import ml_dtypes
    XW_DT = mybir.dt.bfloat16
    XW_NP = ml_dtypes.bfloat16
else:
    XW_DT = F32
    XW_NP = np.float32


def _split_waits(nc, max_waits=1):
    """Walrus in this image allows only ONE sem wait per instruction.
    Hoist extra waits onto injected same-engine NoOps placed immediately
    before the owning instruction (same engine + program order => same
    semantics)."""
    k = 0
    for fn in nc.m.functions:
        for blk in fn.blocks:
            insts = blk.instructions
            out = []
            for inst in insts:
                si = inst.sync_info
                if si is not None and si.on_wait and len(si.on_wait) > max_waits:
                    waits = list(si.on_wait)
                    for w in waits[:-max_waits]:
                        nop = mybir.InstNoOp(name=f"I-wsplit-{k}", ins=[], outs=[])
                        k += 1
                        nop.engine = inst.engine
                        nop.sync_info = bass_rust.SyncInfo(
                            on_wait=[w], on_update=[]
                        )
                        out.append(nop)
                    si.on_wait = waits[-max_waits:]
                    inst.sync_info = si
                out.append(inst)
            if len(out) != len(insts):
                insts.clear()
                insts.extend(out)


def _bilinear_tables(mu):
    """Per-neuron base cell p00, corner offsets (4) in {0,1,36,37}, corner
    weights (4), replicating reference float32 arithmetic exactly."""
    one, half = np.float32(1.0), np.float32(0.5)
    g = np.clip(mu.astype(np.float32), -one, one)
    ix = (g[:, 0] + one) * np.float32(WW * 0.5) - half
    iy = (g[:, 1] + one) * np.float32(HH * 0.5) - half
    x0 = np.floor(ix)
    y0 = np.floor(iy)
    wx1 = ix - x0
    wx0 = one - wx1
    wy1 = iy - y0
    wy0 = one - wy1

    xs = [x0, x0 + one]
    ys = [y0, y0 + one]
    wxs = [wx0, wx1]
    wys = [wy0, wy1]

    x0c = np.clip(x0, 0, WW - 1).astype(np.int64)
    y0c = np.clip(y0, 0, HH - 1).astype(np.int64)
    p00 = y0c * WW + x0c

    offs = np.zeros((4, N), np.int64)
    wgts = np.zeros((4, N), np.float32)
    k = 0
    for a in range(2):          # y corner
        for bb in range(2):     # x corner
            xx, yy = xs[bb], ys[a]
            valid = (xx >= 0) & (xx <= WW - 1) & (yy >= 0) & (yy <= HH - 1)
            xi = np.clip(xx, 0, WW - 1).astype(np.int64)
            yi = np.clip(yy, 0, HH - 1).astype(np.int64)
            offs[k] = yi * WW + xi - p00
            wgts[k] = (wys[a] * wxs[bb]) * valid.astype(np.float32)
            k += 1
    assert offs.min() >= 0 and offs.max() <= 37
    return p00, offs, wgts


def _make_blocks(p00_sorted):
    """Greedy blocks of <=128 sorted neurons with window <= WINMAX."""
    blocks = []  # (start, end) into sorted order
    s = 0
    n = len(p00_sorted)
    while s < n:
        pfirst = p00_sorted[s]
        e = s
        while e < n and e - s < 128 and (p00_sorted[e] - pfirst) + PAD <= WINMAX:
            e += 1
        blocks.append((s, e))
        s = e
    return blocks


def kernel(x, mu, sigma, W, b):
    x = np.ascontiguousarray(x, dtype=np.float32)
    W = np.ascontiguousarray(W, dtype=np.float32)
    b = np.asarray(b, dtype=np.float32)

    p00, offs, wgts = _bilinear_tables(mu)
    order = np.argsort(p00, kind="stable")
    p00s = p00[order]
    blocks = _make_blocks(p00s)
    nblk = len(blocks)

    # per-block host data
    wins, pfirsts, ms = [], [], []
    wparts, mparts = [], []
    biasT = np.zeros((128, 2 * nblk), np.float32)
    for i, (s, e) in enumerate(blocks):
        idx = order[s:e]
        m = e - s
        pfirst = int(p00s[s])
        win = int(p00s[e - 1]) - pfirst + PAD
        ms.append(m)
        pfirsts.append(pfirst)
        wins.append(win)
        # weights: [m,1024] -> [1024,m] -> [8,128,m] -> [128,8,m]
        blkW = W[idx, :].T.reshape(NCHUNK, 128, m).transpose(1, 0, 2)
        wparts.append(np.ascontiguousarray(blkW).reshape(128, NCHUNK * m))
        # mask [128, win] (b-dim broadcast on device)
        mk = np.zeros((128, win), np.float32)
        rel = (p00[idx] - pfirst)  # [m]
        for k in range(4):
            np.add.at(mk[:m], (np.arange(m), rel + offs[k][idx]), wgts[k][idx])
        mparts.append(mk)
        biasT[:m, 2 * i] = b[idx]
        biasT[:m, 2 * i + 1] = b[idx]

    # W packed per block-group, partition-major: each group's DMA moves
    # [128, sum_i 8*m_i] with one long contiguous row per partition.  The
    # last group is kept small so little matmul work trails the final DMA.
    GRPN = 8
    gbounds = [0, 2]
    while gbounds[-1] + GRPN < nblk:
        gbounds.append(gbounds[-1] + GRPN)
    gbounds.append(nblk)
    ngrp = len(gbounds) - 1
    wgrps = []       # per-group [128, gcols] array
    gcol_off = []    # per-block column offset within its group (elements)
    for g in range(ngrp):
        gparts = []
        coff = 0
        for i in range(gbounds[g], gbounds[g + 1]):
            gcol_off.append(coff)
            gparts.append(wparts[i])          # [128, 8*m_i]
            coff += NCHUNK * ms[i]
        wgrps.append(np.ascontiguousarray(np.concatenate(gparts, axis=1)))
    wall = np.ascontiguousarray(np.concatenate(wgrps, axis=1))
    gw_off = np.cumsum([0] + [g.shape[1] for g in wgrps])  # col offset per group
    # all masks packed into one resident [128, sum(win)] tile
    mask_all = np.ascontiguousarray(np.concatenate(mparts, axis=1))
    moffs = np.cumsum([0] + [w for w in wins])
    mtot = int(mask_all.shape[1])

    # ---- build the Bass program (same for all cores) ----
    nc = bass.Bass()
    xs_h = nc.declare_dram_parameter("xs", [128, NCHUNK, BPC, FEATW], XW_DT,
                                     isOutput=False)
    wf_h = nc.declare_dram_parameter("wf", [128, int(wall.shape[1])], XW_DT,
                                     isOutput=False)
    mf_h = nc.declare_dram_parameter("mf", [128, mtot], F32, isOutput=False)
    bt_h = nc.declare_dram_parameter("bt", [128, 2 * nblk], F32, isOutput=False)
    z_h = nc.declare_dram_parameter("z", [128, 2 * nblk], F32, isOutput=True)

    with tile.TileContext(nc) as tc:
        with (
            tc.tile_pool(name="feat", bufs=1) as featp,
            tc.tile_pool(name="wpool", bufs=1) as wpool,
            tc.tile_pool(name="mpool", bufs=1) as mpool,
            tc.tile_pool(name="spool", bufs=4) as spool,
            tc.tile_pool(name="zpool", bufs=1) as zpool,
            tc.tile_pool(name="psum", bufs=1, space="PSUM") as psump,
        ):
            # ---- DMA plan: 3 HWDGE rings, bytes balanced, need-ordered ----
            # feat in 4 two-chunk tiles (long rows); W in 8-block groups.
            feats = [None] * NCHUNK
            wgs = {}

            def load_feat(cpair):
                ft = featp.tile([128, 2, BPC, FEATW], XW_DT,
                                name=f"feat{cpair}")
                feats[2 * cpair] = ft
                return (ft[:], xs_h[:, 2 * cpair:2 * cpair + 2])

            def load_wg(g):
                gcols = int(gw_off[g + 1] - gw_off[g])
                wg = wpool.tile([128, gcols], XW_DT, name=f"wg{g}")
                wgs[g] = wg
                return (wg[:], wf_h[:, int(gw_off[g]):int(gw_off[g + 1])])

            zAll = zpool.tile([128, 2 * nblk], F32)
            nc.vector.memset(zAll[:], 0.0)
            biasT_t = zpool.tile([128, 2 * nblk], F32)
            mask_t = mpool.tile([128, mtot], F32)

            wg_items = [load_wg(g) for g in range(ngrp)]
            sync_items = [wg_items[0], load_feat(1), load_feat(3)]
            scal_items = [load_feat(0), load_feat(2), (mask_t[:], mf_h[:])]
            for g in range(1, ngrp):
                (sync_items if g % 2 == 0 else scal_items).append(wg_items[g])
            scal_items.append((biasT_t[:], bt_h[:]))
            plan = {nc.sync: sync_items, nc.scalar: scal_items}
            maxlen = max(len(v) for v in plan.values())
            for k in range(maxlen):
                for eng, items in plan.items():
                    if k < len(items):
                        dst, srcap = items[k]
                        eng.dma_start(dst, srcap)

            for g in range(ngrp):
                blks = list(range(gbounds[g], gbounds[g + 1]))
                wg = wgs[g]
                pms = {}
                for i in blks:
                    pms[i] = psump.tile([128, 2, wins[i]], F32,
                                        name=f"pm{i}", tag=f"pm{i % 8}")
                for c in range(NCHUNK):
                    for i in blks:
                        m, win, pfirst = ms[i], wins[i], pfirsts[i]
                        o = gcol_off[i]
                        nc.tensor.matmul(
                            pms[i][0:m, :, :],
                            wg[:, o + c * m:o + (c + 1) * m],
                            feats[2 * (c // 2)][:, c % 2, :,
                                                pfirst:pfirst + win],
                            start=(c == 0),
                            stop=(c == NCHUNK - 1),
                        )
                for i in blks:
                    m, win = ms[i], wins[i]
                    mo = int(moffs[i])
                    mk = mask_t[0:m, mo:mo + win].unsqueeze(1).broadcast_to(
                        (m, 2, win)
                    )
                    masked = spool.tile([128, 2, WINMAX], F32, tag="mx")
                    nc.vector.tensor_mul(
                        masked[0:m, :, 0:win], pms[i][0:m, :, :], mk
                    )
                    nc.vector.tensor_reduce(
                        zAll[0:m, 2 * i:2 * i + 2],
                        masked[0:m, :, 0:win],
                        axis=mybir.AxisListType.X,
                        op=mybir.AluOpType.add,
                    )

                # per-group tail: y = z + bias ; out = exp(-relu(-y)) + relu(y)
                sl = slice(2 * blks[0], 2 * (blks[-1] + 1))
                gk = len(blks) * 2
                yt = spool.tile([128, 2 * GRPN], F32, tag="yt")
                nc.vector.tensor_add(yt[:, 0:gk], zAll[:, sl], biasT_t[:, sl])
                rp = spool.tile([128, 2 * GRPN], F32, tag="rp")
                nc.scalar.activation(rp[:, 0:gk], yt[:, 0:gk],
                                     mybir.ActivationFunctionType.Relu)
                rn = spool.tile([128, 2 * GRPN], F32, tag="rn")
                nc.scalar.activation(rn[:, 0:gk], yt[:, 0:gk],
                                     mybir.ActivationFunctionType.Relu,
                                     scale=-1.0)
                ep = spool.tile([128, 2 * GRPN], F32, tag="ep")
                nc.scalar.activation(ep[:, 0:gk], rn[:, 0:gk],
                                     mybir.ActivationFunctionType.Exp,
                                     scale=-1.0)
                ot = spool.tile([128, 2 * GRPN], F32, tag="ot")
                nc.vector.tensor_add(ot[:, 0:gk], ep[:, 0:gk], rp[:, 0:gk])
                nc.sync.dma_start(z_h[:, sl], ot[:, 0:gk])

    _split_waits(nc)

    # ---- run on 8 cores ----
    # xs packed to the exact SBUF layout [128, chunk, batch, FEATW] (zero
    # padded), so each feat DMA moves one long contiguous row per partition.
    xr = x.reshape(B, NCHUNK, 128, P).astype(XW_NP)
    wall_np = wall.astype(XW_NP)
    in_maps = []
    for core in range(NCORES):
        xs_dev = np.zeros((128, NCHUNK, BPC, FEATW), XW_NP)
        xs_dev[:, :, :, :P] = (
            xr[BPC * core:BPC * (core + 1)].transpose(2, 1, 0, 3)
        )
        in_maps.append({
            "xs": xs_dev,
            "wf": wall_np,
            "mf": mask_all,
            "bt": biasT,
        })
    res = run_bass_kernel_spmd(nc, in_maps, core_ids=list(range(NCORES)))

    # ---- assemble ----
    y = np.empty((B, N), np.float32)
    for core in range(NCORES):
        z = res.results[core]["z"]
        for i, (s, e) in enumerate(blocks):
            idx = order[s:e]
            m = e - s
            y[BPC * core, idx] = z[0:m, 2 * i]
            y[BPC * core + 1, idx] = z[0:m, 2 * i + 1]
    return y



# revision 2
# speedup vs baseline: 1.1690x; 1.1690x over previous
"""PoissonGaussianReadout forward on 8 trn2 NeuronCores.

Math (eval mode): each neuron n samples feat[b] (a [36,36,1024] image per
batch, 1024 = C*T channels) bilinearly at a fixed point mu[n], then takes a
per-neuron dot with W[n,:], adds b[n], applies elu(y)+1.

Strategy (v2):
  - Hybrid shard 4x2: 8 cores = 4 batch-groups (4 batches each) x 2 halves
    of the contraction dim D (512 channels each).  Halving the per-core W
    traffic (4.2MB vs 8.4MB replicated) cuts the DMA stream, which the v1
    trace showed to be the critical path (15.2MB @ ~420GB/s aggregate).
  - Sort neurons by bilinear base cell p00 = y0*36+x0; blocks of <=128
    sorted neurons span a window of <=WINMAX flat positions.  One fp32
    matmul per (block, 128-chunk): psum[n, (b,j)] += Wblk^T @ feat-window.
  - Each neuron's 4 bilinear corners live at window offsets
    (p00-pfirst)+{0,1,36,37}; a host-built sparse mask [n, win] holds the
    bilinear weights there.  DVE scalar_tensor_tensor fuses mask-mult and
    window-reduce: z[n, b] = sum_j psum[n,b,j]*mask[n,j] in one pass.
  - Cores emit LINEAR partial sums z; the host adds the two D-halves,
    adds bias and applies elu(y)+1 on the tiny [16,4096] output.
  - DMA is need-ordered across the two HWDGE queues (SP + Act): feat
    chunks (split in halves) arrive progressively, W block-groups arrive
    just-in-time behind PE consumption, per-group mask slices ride along
    so DVE never stalls on the mask.

The block structure / masks depend on mu, which is known when kernel() is
called; the Bass program is traced fresh per call, so they are baked in as
compile-time constants (correct for any input values).
"""
import sys
sys.path.insert(0, "/opt/trn_rl_repo")

import numpy as np

from concourse import bass, mybir, tile
from concourse.bass_utils import run_bass_kernel_spmd
import bass_rust

# problem constants
B, C, T, HH, WW = 16, 64, 16, 36, 36
N, D = 4096, C * T             # 4096 neurons, 1024 input dim
P = HH * WW                    # 1296 flat positions
NCORES = 8
NBG = 4                        # batch groups
NDH = 2                        # D halves
BPC = B // NBG                 # batches per core = 4
DH = D // NDH                  # channels per core = 512
NCHUNK = DH // 128             # 4 contraction chunks per core
PAD = 38                       # max corner offset (37) + 1
WINMAX = 128                   # psum bank: BPC*WIN <= 512 fp32
FEATW = P + PAD                # padded feat width per (chunk, batch)
GRPN = 4                       # blocks per W DMA group

F32 = mybir.dt.float32

import ml_dtypes
XW_DT = mybir.dt.bfloat16
XW_NP = ml_dtypes.bfloat16


def _split_waits(nc, max_waits=1):
    """Walrus in this image allows only ONE sem wait per instruction.
    Hoist extra waits onto injected same-engine NoOps placed immediately
    before the owning instruction (same engine + program order => same
    semantics)."""
    k = 0
    for fn in nc.m.functions:
        for blk in fn.blocks:
            insts = blk.instructions
            out = []
            for inst in insts:
                si = inst.sync_info
                if si is not None and si.on_wait and len(si.on_wait) > max_waits:
                    waits = list(si.on_wait)
                    for w in waits[:-max_waits]:
                        nop = mybir.InstNoOp(name=f"I-wsplit-{k}", ins=[], outs=[])
                        k += 1
                        nop.engine = inst.engine
                        nop.sync_info = bass_rust.SyncInfo(
                            on_wait=[w], on_update=[]
                        )
                        out.append(nop)
                    si.on_wait = waits[-max_waits:]
                    inst.sync_info = si
                out.append(inst)
            if len(out) != len(insts):
                insts.clear()
                insts.extend(out)


def _bilinear_tables(mu):
    """Per-neuron base cell p00, corner offsets (4) in {0,1,36,37}, corner
    weights (4), replicating reference float32 arithmetic exactly."""
    one, half = np.float32(1.0), np.float32(0.5)
    g = np.clip(mu.astype(np.float32), -one, one)
    ix = (g[:, 0] + one) * np.float32(WW * 0.5) - half
    iy = (g[:, 1] + one) * np.float32(HH * 0.5) - half
    x0 = np.floor(ix)
    y0 = np.floor(iy)
    wx1 = ix - x0
    wx0 = one - wx1
    wy1 = iy - y0
    wy0 = one - wy1

    xs = [x0, x0 + one]
    ys = [y0, y0 + one]
    wxs = [wx0, wx1]
    wys = [wy0, wy1]

    x0c = np.clip(x0, 0, WW - 1).astype(np.int64)
    y0c = np.clip(y0, 0, HH - 1).astype(np.int64)
    p00 = y0c * WW + x0c

    offs = np.zeros((4, N), np.int64)
    wgts = np.zeros((4, N), np.float32)
    k = 0
    for a in range(2):          # y corner
        for bb in range(2):     # x corner
            xx, yy = xs[bb], ys[a]
            valid = (xx >= 0) & (xx <= WW - 1) & (yy >= 0) & (yy <= HH - 1)
            xi = np.clip(xx, 0, WW - 1).astype(np.int64)
            yi = np.clip(yy, 0, HH - 1).astype(np.int64)
            offs[k] = yi * WW + xi - p00
            wgts[k] = (wys[a] * wxs[bb]) * valid.astype(np.float32)
            k += 1
    assert offs.min() >= 0 and offs.max() <= 37
    return p00, offs, wgts


def _make_blocks(p00_sorted):
    """Greedy blocks of <=128 sorted neurons with window <= WINMAX."""
    blocks = []  # (start, end) into sorted order
    s = 0
    n = len(p00_sorted)
    while s < n:
        pfirst = p00_sorted[s]
        e = s
        while e < n and e - s < 128 and (p00_sorted[e] - pfirst) + PAD <= WINMAX:
            e += 1
        blocks.append((s, e))
        s = e
    return blocks


def kernel(x, mu, sigma, W, b):
    x = np.ascontiguousarray(x, dtype=np.float32)
    W = np.ascontiguousarray(W, dtype=np.float32)
    b = np.asarray(b, dtype=np.float32)

    p00, offs, wgts = _bilinear_tables(mu)
    order = np.argsort(p00, kind="stable")
    p00s = p00[order]
    blocks = _make_blocks(p00s)
    nblk = len(blocks)

    # per-block host data
    wins, pfirsts, ms = [], [], []
    mparts = []
    wparts = [[], []]  # per D-half: per-block [128, NCHUNK*m]
    for i, (s, e) in enumerate(blocks):
        idx = order[s:e]
        m = e - s
        pfirst = int(p00s[s])
        win = int(p00s[e - 1]) - pfirst + PAD
        ms.append(m)
        pfirsts.append(pfirst)
        wins.append(win)
        for dh in range(NDH):
            # weights: [m,512] -> [512,m] -> [4,128,m] -> [128,4,m]
            blkW = (W[idx, dh * DH:(dh + 1) * DH].T
                    .reshape(NCHUNK, 128, m).transpose(1, 0, 2))
            wparts[dh].append(np.ascontiguousarray(blkW).reshape(128, NCHUNK * m))
        # mask [128, win]
        mk = np.zeros((128, win), np.float32)
        rel = (p00[idx] - pfirst)  # [m]
        for k in range(4):
            np.add.at(mk[:m], (np.arange(m), rel + offs[k][idx]), wgts[k][idx])
        mparts.append(mk)

    # W packed per block-group, partition-major; first group small so the
    # PE can start early, the rest sized GRPN.
    gbounds = [0, min(2, nblk)]
    while gbounds[-1] + GRPN < nblk:
        gbounds.append(gbounds[-1] + GRPN)
    if gbounds[-1] < nblk:
        gbounds.append(nblk)
    ngrp = len(gbounds) - 1
    gcol_off = []    # per-block column offset within its group (elements)
    gw_off = [0]     # per-group column offset in the packed W
    for g in range(ngrp):
        coff = 0
        for i in range(gbounds[g], gbounds[g + 1]):
            gcol_off.append(coff)
            coff += NCHUNK * ms[i]
        gw_off.append(gw_off[-1] + coff)
    walls = []
    for dh in range(NDH):
        walls.append(np.ascontiguousarray(np.concatenate(wparts[dh], axis=1)))
    # all masks packed into one resident [128, sum(win)] tile
    mask_all = np.ascontiguousarray(np.concatenate(mparts, axis=1))
    moffs = np.cumsum([0] + [w for w in wins])
    mtot = int(mask_all.shape[1])

    # ---- build the Bass program (same for all cores) ----
    nc = bass.Bass()
    xs_h = nc.declare_dram_parameter("xs", [128, NCHUNK, BPC, FEATW], XW_DT,
                                     isOutput=False)
    wf_h = nc.declare_dram_parameter("wf", [128, int(walls[0].shape[1])], XW_DT,
                                     isOutput=False)
    mf_h = nc.declare_dram_parameter("mf", [128, mtot], F32, isOutput=False)
    z_h = nc.declare_dram_parameter("z", [128, BPC * nblk], F32, isOutput=True)

    with tile.TileContext(nc) as tc:
        with (
            tc.tile_pool(name="feat", bufs=1) as featp,
            tc.tile_pool(name="wpool", bufs=1) as wpool,
            tc.tile_pool(name="mpool", bufs=1) as mpool,
            tc.tile_pool(name="spool", bufs=4) as spool,
            tc.tile_pool(name="zpool", bufs=1) as zpool,
            tc.tile_pool(name="psum", bufs=1, space="PSUM") as psump,
        ):
            # one resident feat tile per chunk; each filled by 2 half-DMAs
            fts = [featp.tile([128, 1, BPC, FEATW], XW_DT, name=f"feat{c}")
                   for c in range(NCHUNK)]
            mask_t = mpool.tile([128, mtot], F32)
            zAll = zpool.tile([128, BPC * nblk], F32)
            wgs = {}
            for g in range(ngrp):
                gcols = int(gw_off[g + 1] - gw_off[g])
                wgs[g] = wpool.tile([128, gcols], XW_DT, name=f"wg{g}")

            # ---- DMA plan: need-ordered rounds on the 2 HWDGE queues ----
            # feat chunk halves alternate queues; W group g (+ its mask
            # slice) goes to queue g%2, interleaved between feat chunks.
            def feat_half(c, h):
                return (fts[c][:, :, 2 * h:2 * h + 2, :],
                        xs_h[:, c:c + 1, 2 * h:2 * h + 2, :])

            def wg_item(g):
                return (wgs[g][:], wf_h[:, int(gw_off[g]):int(gw_off[g + 1])])

            def mask_item(g):
                lo = int(moffs[gbounds[g]])
                hi = int(moffs[gbounds[g + 1]])
                return (mask_t[:, lo:hi], mf_h[:, lo:hi])

            sync_items = [[feat_half(0, 0)]]
            scal_items = [[feat_half(0, 1)]]
            nextq = [sync_items, scal_items]
            qfill = [1, 1]  # rounds consumed
            # round-robin: after feat0, emit W0+m0 / W1+m1, then feat1
            # halves, W2/W3, feat2, W4/W5, feat3, W6/W7, then the rest.
            emit = []
            wq = 0
            gi = 0
            for c in range(1, NCHUNK):
                for _ in range(2):  # two W groups between feat chunks
                    if gi < ngrp:
                        emit.append(("w", gi))
                        gi += 1
                emit.append(("f", c))
            while gi < ngrp:
                emit.append(("w", gi))
                gi += 1
            for kind, v in emit:
                if kind == "f":
                    sync_items.append([feat_half(v, 0)])
                    scal_items.append([feat_half(v, 1)])
                else:
                    (sync_items if (v % 2 == 0) else scal_items).append(
                        [wg_item(v), mask_item(v)]
                    )
            plan = {nc.sync: sync_items, nc.scalar: scal_items}
            maxlen = max(len(v) for v in plan.values())
            for k in range(maxlen):
                for eng, items in plan.items():
                    if k < len(items):
                        for dst, srcap in items[k]:
                            eng.dma_start(dst, srcap)

            ADD = mybir.AluOpType.add
            MULT = mybir.AluOpType.mult
            for g in range(ngrp):
                blks = list(range(gbounds[g], gbounds[g + 1]))
                wg = wgs[g]
                pms = {}
                for i in blks:
                    pms[i] = psump.tile([128, BPC, wins[i]], F32,
                                        name=f"pm{i}", tag=f"pm{i % 8}")
                for c in range(NCHUNK):
                    for i in blks:
                        m, win, pfirst = ms[i], wins[i], pfirsts[i]
                        o = gcol_off[i]
                        nc.tensor.matmul(
                            pms[i][0:m, :, :],
                            wg[:, o + c * m:o + (c + 1) * m],
                            fts[c][:, 0, :, pfirst:pfirst + win],
                            start=(c == 0),
                            stop=(c == NCHUNK - 1),
                        )
                # fused mask-mult + window-reduce on DVE, one per batch
                for i in blks:
                    m, win = ms[i], wins[i]
                    mo = int(moffs[i])
                    for bb in range(BPC):
                        sc = spool.tile([128, WINMAX], F32, tag=f"sc{bb}")
                        nc.vector.scalar_tensor_tensor(
                            sc[0:m, 0:win],
                            pms[i][0:m, bb, :],
                            0.0,
                            mask_t[0:m, mo:mo + win],
                            ADD,
                            MULT,
                            accum_out=zAll[0:m, BPC * i + bb:BPC * i + bb + 1],
                        )
            # single output store once every partial is in place
            nc.scalar.dma_start(z_h[:], zAll[:])

    _split_waits(nc)

    # ---- run on 8 cores: core id = bg*2 + dh ----
    xr = x.reshape(B, D // 128, 128, P).astype(XW_NP)
    walls_np = [w.astype(XW_NP) for w in walls]
    in_maps = []
    for core in range(NCORES):
        bg, dh = core // NDH, core % NDH
        xs_dev = np.zeros((128, NCHUNK, BPC, FEATW), XW_NP)
        # [128, chunk, batch, P]
        xs_dev[:, :, :, :P] = (
            xr[BPC * bg:BPC * (bg + 1), NCHUNK * dh:NCHUNK * (dh + 1)]
            .transpose(2, 1, 0, 3)
        )
        in_maps.append({
            "xs": xs_dev,
            "wf": walls_np[dh],
            "mf": mask_all,
        })
    res = run_bass_kernel_spmd(nc, in_maps, core_ids=list(range(NCORES)))

    # ---- assemble: add D-halves, bias, elu(y)+1 ----
    y = np.empty((B, N), np.float32)
    for bg in range(NBG):
        z = res.results[NDH * bg]["z"] + res.results[NDH * bg + 1]["z"]
        for i, (s, e) in enumerate(blocks):
            idx = order[s:e]
            m = e - s
            y[BPC * bg:BPC * (bg + 1), idx] = z[0:m, BPC * i:BPC * (i + 1)].T
    y += b
    return np.where(y > 0, y + np.float32(1.0),
                    np.exp(np.minimum(y, np.float32(0.0)))).astype(np.float32)


# revision 4
# speedup vs baseline: 1.2650x; 1.0821x over previous
"""PoissonGaussianReadout forward on 8 trn2 NeuronCores.

Math (eval mode): each neuron n samples feat[b] (a [36,36,1024] image per
batch, 1024 = C*T channels) bilinearly at a fixed point mu[n], then takes a
per-neuron dot with W[n,:], adds b[n], applies elu(y)+1.

Strategy (v2):
  - Hybrid shard 4x2: 8 cores = 4 batch-groups (4 batches each) x 2 halves
    of the contraction dim D (512 channels each).  Halving the per-core W
    traffic (4.2MB vs 8.4MB replicated) cuts the DMA stream, which the v1
    trace showed to be the critical path (15.2MB @ ~420GB/s aggregate).
  - Sort neurons by bilinear base cell p00 = y0*36+x0; blocks of <=128
    sorted neurons span a window of <=WINMAX flat positions.  One fp32
    matmul per (block, 128-chunk): psum[n, (b,j)] += Wblk^T @ feat-window.
  - Each neuron's 4 bilinear corners live at window offsets
    (p00-pfirst)+{0,1,36,37}; a host-built sparse mask [n, win] holds the
    bilinear weights there.  DVE scalar_tensor_tensor fuses mask-mult and
    window-reduce: z[n, b] = sum_j psum[n,b,j]*mask[n,j] in one pass.
  - Cores emit LINEAR partial sums z; the host adds the two D-halves,
    adds bias and applies elu(y)+1 on the tiny [16,4096] output.
  - DMA is need-ordered across the two HWDGE queues (SP + Act): feat
    chunks (split in halves) arrive progressively, W block-groups arrive
    just-in-time behind PE consumption, per-group mask slices ride along
    so DVE never stalls on the mask.

The block structure / masks depend on mu, which is known when kernel() is
called; the Bass program is traced fresh per call, so they are baked in as
compile-time constants (correct for any input values).
"""
import sys
sys.path.insert(0, "/opt/trn_rl_repo")

import numpy as np

from concourse import bass, mybir, tile
from concourse.bass_utils import run_bass_kernel_spmd
import bass_rust

# problem constants
B, C, T, HH, WW = 16, 64, 16, 36, 36
N, D = 4096, C * T             # 4096 neurons, 1024 input dim
P = HH * WW                    # 1296 flat positions
NCORES = 8
NBG = 4                        # batch groups
NDH = 2                        # D halves
BPC = B // NBG                 # batches per core = 4
DH = D // NDH                  # channels per core = 512
NCHUNK = DH // 128             # 4 contraction chunks per core
PAD = 38                       # max corner offset (37) + 1
WINMAX = 128                   # psum bank: BPC*WIN <= 512 fp32
FEATW = P + PAD                # padded feat width per (chunk, batch)
GRPN = 4                       # blocks per W DMA group

F32 = mybir.dt.float32

import ml_dtypes
XW_DT = mybir.dt.bfloat16
XW_NP = ml_dtypes.bfloat16


def _split_waits(nc, max_waits=1):
    """Walrus in this image allows only ONE sem wait per instruction.
    Hoist extra waits onto injected same-engine NoOps placed immediately
    before the owning instruction (same engine + program order => same
    semantics)."""
    k = 0
    for fn in nc.m.functions:
        for blk in fn.blocks:
            insts = blk.instructions
            out = []
            for inst in insts:
                si = inst.sync_info
                if si is not None and si.on_wait and len(si.on_wait) > max_waits:
                    waits = list(si.on_wait)
                    for w in waits[:-max_waits]:
                        nop = mybir.InstNoOp(name=f"I-wsplit-{k}", ins=[], outs=[])
                        k += 1
                        nop.engine = inst.engine
                        nop.sync_info = bass_rust.SyncInfo(
                            on_wait=[w], on_update=[]
                        )
                        out.append(nop)
                    si.on_wait = waits[-max_waits:]
                    inst.sync_info = si
                out.append(inst)
            if len(out) != len(insts):
                insts.clear()
                insts.extend(out)


def _bilinear_tables(mu):
    """Per-neuron base cell p00, corner offsets (4) in {0,1,36,37}, corner
    weights (4), replicating reference float32 arithmetic exactly."""
    one, half = np.float32(1.0), np.float32(0.5)
    g = np.clip(mu.astype(np.float32), -one, one)
    ix = (g[:, 0] + one) * np.float32(WW * 0.5) - half
    iy = (g[:, 1] + one) * np.float32(HH * 0.5) - half
    x0 = np.floor(ix)
    y0 = np.floor(iy)
    wx1 = ix - x0
    wx0 = one - wx1
    wy1 = iy - y0
    wy0 = one - wy1

    xs = [x0, x0 + one]
    ys = [y0, y0 + one]
    wxs = [wx0, wx1]
    wys = [wy0, wy1]

    x0c = np.clip(x0, 0, WW - 1).astype(np.int64)
    y0c = np.clip(y0, 0, HH - 1).astype(np.int64)
    p00 = y0c * WW + x0c

    offs = np.zeros((4, N), np.int64)
    wgts = np.zeros((4, N), np.float32)
    k = 0
    for a in range(2):          # y corner
        for bb in range(2):     # x corner
            xx, yy = xs[bb], ys[a]
            valid = (xx >= 0) & (xx <= WW - 1) & (yy >= 0) & (yy <= HH - 1)
            xi = np.clip(xx, 0, WW - 1).astype(np.int64)
            yi = np.clip(yy, 0, HH - 1).astype(np.int64)
            offs[k] = yi * WW + xi - p00
            wgts[k] = (wys[a] * wxs[bb]) * valid.astype(np.float32)
            k += 1
    assert offs.min() >= 0 and offs.max() <= 37
    return p00, offs, wgts


def _make_blocks(p00_sorted):
    """Greedy blocks of <=128 sorted neurons with window <= WINMAX."""
    blocks = []  # (start, end) into sorted order
    s = 0
    n = len(p00_sorted)
    while s < n:
        pfirst = p00_sorted[s]
        e = s
        while e < n and e - s < 128 and (p00_sorted[e] - pfirst) + PAD <= WINMAX:
            e += 1
        blocks.append((s, e))
        s = e
    return blocks


def kernel(x, mu, sigma, W, b):
    x = np.ascontiguousarray(x, dtype=np.float32)
    W = np.ascontiguousarray(W, dtype=np.float32)
    b = np.asarray(b, dtype=np.float32)

    p00, offs, wgts = _bilinear_tables(mu)
    order = np.argsort(p00, kind="stable")
    p00s = p00[order]
    blocks = _make_blocks(p00s)
    nblk = len(blocks)

    # per-block host data
    wins, pfirsts, ms = [], [], []
    mparts = []
    wparts = [[], []]  # per D-half: per-block [128, NCHUNK*m]
    for i, (s, e) in enumerate(blocks):
        idx = order[s:e]
        m = e - s
        pfirst = int(p00s[s])
        win = int(p00s[e - 1]) - pfirst + PAD
        ms.append(m)
        pfirsts.append(pfirst)
        wins.append(win)
        for dh in range(NDH):
            # weights: [m,512] -> [512,m] -> [4,128,m] -> [128,4,m]
            blkW = (W[idx, dh * DH:(dh + 1) * DH].T
                    .reshape(NCHUNK, 128, m).transpose(1, 0, 2))
            wparts[dh].append(np.ascontiguousarray(blkW).reshape(128, NCHUNK * m))
        # mask [128, win]
        mk = np.zeros((128, win), np.float32)
        rel = (p00[idx] - pfirst)  # [m]
        for k in range(4):
            np.add.at(mk[:m], (np.arange(m), rel + offs[k][idx]), wgts[k][idx])
        mparts.append(mk)

    # W packed per block-group, partition-major; first group small so the
    # PE can start early, the rest sized GRPN.
    gbounds = [0, min(2, nblk)]
    while gbounds[-1] + GRPN < nblk:
        gbounds.append(gbounds[-1] + GRPN)
    if gbounds[-1] < nblk:
        gbounds.append(nblk)
    ngrp = len(gbounds) - 1
    gcol_off = []    # per-block column offset within its group (elements)
    gw_off = [0]     # per-group column offset in the packed W
    for g in range(ngrp):
        coff = 0
        for i in range(gbounds[g], gbounds[g + 1]):
            gcol_off.append(coff)
            coff += NCHUNK * ms[i]
        gw_off.append(gw_off[-1] + coff)
    walls = []
    for dh in range(NDH):
        walls.append(np.ascontiguousarray(np.concatenate(wparts[dh], axis=1)))
    # all masks packed into one resident [128, sum(win)] tile
    mask_all = np.ascontiguousarray(np.concatenate(mparts, axis=1))
    moffs = np.cumsum([0] + [w for w in wins])
    mtot = int(mask_all.shape[1])

    # ---- build the Bass program (same for all cores) ----
    nc = bass.Bass()
    xs_h = nc.declare_dram_parameter("xs", [128, NCHUNK, BPC, FEATW], XW_DT,
                                     isOutput=False)
    wf_h = nc.declare_dram_parameter("wf", [128, int(walls[0].shape[1])], XW_DT,
                                     isOutput=False)
    mf_h = nc.declare_dram_parameter("mf", [128, mtot], F32, isOutput=False)
    z_h = nc.declare_dram_parameter("z", [128, BPC * nblk], F32, isOutput=True)

    with tile.TileContext(nc) as tc:
        with (
            tc.tile_pool(name="feat", bufs=1) as featp,
            tc.tile_pool(name="wpool", bufs=1) as wpool,
            tc.tile_pool(name="mpool", bufs=1) as mpool,
            tc.tile_pool(name="spool", bufs=4) as spool,
            tc.tile_pool(name="zpool", bufs=1) as zpool,
            tc.tile_pool(name="psum", bufs=1, space="PSUM") as psump,
        ):
            # one resident feat tile per chunk; each filled by 2 half-DMAs
            fts = [featp.tile([128, 1, BPC, FEATW], XW_DT, name=f"feat{c}")
                   for c in range(NCHUNK)]
            mask_t = mpool.tile([128, mtot], F32)
            zAll = zpool.tile([128, BPC * nblk], F32)
            wgs = {}
            for g in range(ngrp):
                gcols = int(gw_off[g + 1] - gw_off[g])
                wgs[g] = wpool.tile([128, gcols], XW_DT, name=f"wg{g}")

            # ---- DMA plan: need-ordered rounds on the 2 HWDGE queues ----
            # feat chunk halves alternate queues; W group g (+ its mask
            # slice) goes to queue g%2, interleaved between feat chunks.
            def feat_half(c, h):
                return (fts[c][:, :, 2 * h:2 * h + 2, :],
                        xs_h[:, c:c + 1, 2 * h:2 * h + 2, :])

            def wg_item(g):
                return (wgs[g][:], wf_h[:, int(gw_off[g]):int(gw_off[g + 1])])

            def mask_item(g):
                lo = int(moffs[gbounds[g]])
                hi = int(moffs[gbounds[g + 1]])
                return (mask_t[:, lo:hi], mf_h[:, lo:hi])

            # All feat first (group-major matmul order consumes every chunk
            # within the first group, so feat arrival gates the whole PE
            # stream), then W groups + their mask slices alternating queues.
            sync_items = [[feat_half(c, 0)] for c in range(NCHUNK)]
            scal_items = [[feat_half(c, 1)] for c in range(NCHUNK)]
            for g in range(ngrp):
                (sync_items if (g % 2 == 0) else scal_items).append(
                    [wg_item(g), mask_item(g)]
                )
            plan = {nc.sync: sync_items, nc.scalar: scal_items}
            maxlen = max(len(v) for v in plan.values())
            for k in range(maxlen):
                for eng, items in plan.items():
                    if k < len(items):
                        for dst, srcap in items[k]:
                            eng.dma_start(dst, srcap)

            ADD = mybir.AluOpType.add
            MULT = mybir.AluOpType.mult
            for g in range(ngrp):
                blks = list(range(gbounds[g], gbounds[g + 1]))
                wg = wgs[g]
                pms = {}
                for i in blks:
                    pms[i] = psump.tile([128, BPC, wins[i]], F32,
                                        name=f"pm{i}", tag=f"pm{i % 8}")
                for c in range(NCHUNK):
                    for i in blks:
                        m, win, pfirst = ms[i], wins[i], pfirsts[i]
                        o = gcol_off[i]
                        nc.tensor.matmul(
                            pms[i][0:m, :, :],
                            wg[:, o + c * m:o + (c + 1) * m],
                            fts[c][:, 0, :, pfirst:pfirst + win],
                            start=(c == 0),
                            stop=(c == NCHUNK - 1),
                        )
                # mask-mult (psum f32 -> bf16 scratch) + window-reduce; the
                # bf16 intermediate halves the reduce pass's read time.
                for i in blks:
                    m, win = ms[i], wins[i]
                    mo = int(moffs[i])
                    mk = mask_t[0:m, mo:mo + win].unsqueeze(1).broadcast_to(
                        (m, BPC, win)
                    )
                    sc = spool.tile([128, BPC, WINMAX], XW_DT, tag=f"sc{i % 4}")
                    nc.vector.tensor_mul(
                        sc[0:m, :, 0:win], pms[i][0:m, :, :], mk
                    )
                    nc.vector.tensor_reduce(
                        zAll[0:m, BPC * i:BPC * (i + 1)],
                        sc[0:m, :, 0:win],
                        axis=mybir.AxisListType.X,
                        op=mybir.AluOpType.add,
                    )
            # single output store once every partial is in place
            nc.scalar.dma_start(z_h[:], zAll[:])

    _split_waits(nc)

    # ---- run on 8 cores: core id = bg*2 + dh ----
    xr = x.reshape(B, D // 128, 128, P).astype(XW_NP)
    walls_np = [w.astype(XW_NP) for w in walls]
    in_maps = []
    for core in range(NCORES):
        bg, dh = core // NDH, core % NDH
        xs_dev = np.zeros((128, NCHUNK, BPC, FEATW), XW_NP)
        # [128, chunk, batch, P]
        xs_dev[:, :, :, :P] = (
            xr[BPC * bg:BPC * (bg + 1), NCHUNK * dh:NCHUNK * (dh + 1)]
            .transpose(2, 1, 0, 3)
        )
        in_maps.append({
            "xs": xs_dev,
            "wf": walls_np[dh],
            "mf": mask_all,
        })
    res = run_bass_kernel_spmd(nc, in_maps, core_ids=list(range(NCORES)))

    # ---- assemble: add D-halves, bias, elu(y)+1 ----
    y = np.empty((B, N), np.float32)
    for bg in range(NBG):
        z = res.results[NDH * bg]["z"] + res.results[NDH * bg + 1]["z"]
        for i, (s, e) in enumerate(blocks):
            idx = order[s:e]
            m = e - s
            y[BPC * bg:BPC * (bg + 1), idx] = z[0:m, BPC * i:BPC * (i + 1)].T
    y += b
    return np.where(y > 0, y + np.float32(1.0),
                    np.exp(np.minimum(y, np.float32(0.0)))).astype(np.float32)


# revision 6
# speedup vs baseline: 1.4439x; 1.1414x over previous
"""PoissonGaussianReadout forward on 8 trn2 NeuronCores.

Math (eval mode): each neuron n samples feat[b] (a [36,36,1024] image per
batch, 1024 = C*T channels) bilinearly at a fixed point mu[n], then takes a
per-neuron dot with W[n,:], adds b[n], applies elu(y)+1.

Strategy (v4):
  - Hybrid shard 4x2: 8 cores = 4 batch-groups (4 batches each) x 2 halves
    of the contraction dim D (512 channels each).  Cores emit LINEAR
    partial sums; the host adds the halves, bias, and elu on [16,4096].
  - fp8(e4m3) x and W with DoubleRow matmuls: x uses one global scale, W a
    per-neuron scale; both dequant factors fold into the (per-neuron) mask.
    Halves both the DMA stream and the PE time vs bf16.
  - Sort neurons by bilinear base cell p00; blocks of <=128 sorted neurons
    span a window of <=WINMAX flat positions.  Two DoubleRow matmuls per
    block (256-channel subtile pairs): psum[n,(b,j)] += Wblk^T @ feat-win.
  - Each neuron's 4 bilinear corners live at window offsets
    (p00-pfirst)+{0,1,36,37}; a host-built sparse mask [n, win] holds the
    bilinear weights (pre-divided by the fp8 scales).  The mask-multiply +
    window-reduce runs as scalar_tensor_tensor with accum, split between
    the DVE and GpSimd engines (blocks i%3==2 go to GpSimd).
  - DMA is need-ordered on the two HWDGE queues: feat chunk halves first,
    W block-groups + their mask slices just-in-time behind PE consumption.
"""
import sys
sys.path.insert(0, "/opt/trn_rl_repo")

import numpy as np

from concourse import bass, mybir, tile
from concourse.bass_utils import run_bass_kernel_spmd
import bass_rust

# problem constants
B, C, T, HH, WW = 16, 64, 16, 36, 36
N, D = 4096, C * T             # 4096 neurons, 1024 input dim
P = HH * WW                    # 1296 flat positions
NCORES = 8
NBG = 4                        # batch groups
NDH = 2                        # D halves
BPC = B // NBG                 # batches per core = 4
DH = D // NDH                  # channels per core = 512
NC2 = DH // 256                # 2 double-subtile (256-chan) passes per core
PAD = 38                       # max corner offset (37) + 1
WINMAX = 128                   # psum bank: BPC*WIN <= 512 fp32
FEATW = P + PAD                # padded feat width per (chunk, batch)
GRPN = 4                       # blocks per W DMA group

F32 = mybir.dt.float32

import ml_dtypes
F8_DT = mybir.dt.float8e4
F8_NP = ml_dtypes.float8_e4m3   # max normal 240
F8_CAP = np.float32(224.0)


def _split_waits(nc, max_waits=1):
    """Walrus in this image allows only ONE sem wait per instruction.
    Hoist extra waits onto injected same-engine NoOps placed immediately
    before the owning instruction (same engine + program order => same
    semantics)."""
    k = 0
    for fn in nc.m.functions:
        for blk in fn.blocks:
            insts = blk.instructions
            out = []
            for inst in insts:
                si = inst.sync_info
                if si is not None and si.on_wait and len(si.on_wait) > max_waits:
                    waits = list(si.on_wait)
                    for w in waits[:-max_waits]:
                        nop = mybir.InstNoOp(name=f"I-wsplit-{k}", ins=[], outs=[])
                        k += 1
                        nop.engine = inst.engine
                        nop.sync_info = bass_rust.SyncInfo(
                            on_wait=[w], on_update=[]
                        )
                        out.append(nop)
                    si.on_wait = waits[-max_waits:]
                    inst.sync_info = si
                out.append(inst)
            if len(out) != len(insts):
                insts.clear()
                insts.extend(out)


def _bilinear_tables(mu):
    """Per-neuron base cell p00, corner offsets (4) in {0,1,36,37}, corner
    weights (4), replicating reference float32 arithmetic exactly."""
    one, half = np.float32(1.0), np.float32(0.5)
    g = np.clip(mu.astype(np.float32), -one, one)
    ix = (g[:, 0] + one) * np.float32(WW * 0.5) - half
    iy = (g[:, 1] + one) * np.float32(HH * 0.5) - half
    x0 = np.floor(ix)
    y0 = np.floor(iy)
    wx1 = ix - x0
    wx0 = one - wx1
    wy1 = iy - y0
    wy0 = one - wy1

    xs = [x0, x0 + one]
    ys = [y0, y0 + one]
    wxs = [wx0, wx1]
    wys = [wy0, wy1]

    x0c = np.clip(x0, 0, WW - 1).astype(np.int64)
    y0c = np.clip(y0, 0, HH - 1).astype(np.int64)
    p00 = y0c * WW + x0c

    offs = np.zeros((4, N), np.int64)
    wgts = np.zeros((4, N), np.float32)
    k = 0
    for a in range(2):          # y corner
        for bb in range(2):     # x corner
            xx, yy = xs[bb], ys[a]
            valid = (xx >= 0) & (xx <= WW - 1) & (yy >= 0) & (yy <= HH - 1)
            xi = np.clip(xx, 0, WW - 1).astype(np.int64)
            yi = np.clip(yy, 0, HH - 1).astype(np.int64)
            offs[k] = yi * WW + xi - p00
            wgts[k] = (wys[a] * wxs[bb]) * valid.astype(np.float32)
            k += 1
    assert offs.min() >= 0 and offs.max() <= 37
    return p00, offs, wgts


def _make_blocks(p00_sorted):
    """Greedy blocks of <=128 sorted neurons with window <= WINMAX."""
    blocks = []  # (start, end) into sorted order
    s = 0
    n = len(p00_sorted)
    while s < n:
        pfirst = p00_sorted[s]
        e = s
        while e < n and e - s < 128 and (p00_sorted[e] - pfirst) + PAD <= WINMAX:
            e += 1
        blocks.append((s, e))
        s = e
    return blocks


def kernel(x, mu, sigma, W, b):
    x = np.ascontiguousarray(x, dtype=np.float32)
    W = np.ascontiguousarray(W, dtype=np.float32)
    b = np.asarray(b, dtype=np.float32)

    p00, offs, wgts = _bilinear_tables(mu)
    order = np.argsort(p00, kind="stable")
    p00s = p00[order]
    blocks = _make_blocks(p00s)
    nblk = len(blocks)

    # ---- fp8 quantization: global x scale, per-neuron W scale ----
    sx = F8_CAP / np.float32(max(np.abs(x).max(), 1e-30))
    sw = F8_CAP / np.maximum(np.abs(W).max(axis=1), 1e-30).astype(np.float32)
    Wq = (W * sw[:, None]).astype(F8_NP)    # [N, D]
    dequant = 1.0 / (sw * sx)               # [N] folded into the mask

    # per-block host data
    wins, pfirsts, ms, sblk = [], [], [], []
    mparts = []
    for i, (s, e) in enumerate(blocks):
        idx = order[s:e]
        m = e - s
        pfirst = int(p00s[s])
        win = int(p00s[e - 1]) - pfirst + PAD
        ms.append(m)
        pfirsts.append(pfirst)
        wins.append(win)
        sblk.append(s)
        # mask [128, win], fp8 dequant folded in
        mk = np.zeros((128, win), np.float32)
        rel = (p00[idx] - pfirst)  # [m]
        for k in range(4):
            np.add.at(mk[:m], (np.arange(m), rel + offs[k][idx]),
                      wgts[k][idx] * dequant[idx])
        mparts.append(mk)
    sblk.append(N)

    # W groups: first small so the PE can start early, the rest sized GRPN
    gbounds = [0, min(2, nblk)]
    while gbounds[-1] + GRPN < nblk:
        gbounds.append(gbounds[-1] + GRPN)
    if gbounds[-1] < nblk:
        gbounds.append(nblk)
    ngrp = len(gbounds) - 1

    # W packed [128, NC2, 2, N] per D-half; neuron column = sorted order, so
    # group g's DMA is the contiguous column slice [sblk[gb[g]], sblk[gb[g+1]])
    Ws = Wq[order]                          # [N, D] sorted
    walls = []
    for dh in range(NDH):
        wl = (Ws[:, dh * DH:(dh + 1) * DH].T        # [512, N]
              .reshape(NC2, 2, 128, N).transpose(2, 0, 1, 3))
        walls.append(np.ascontiguousarray(wl))      # [128, NC2, 2, N]
    mask_all = np.ascontiguousarray(np.concatenate(mparts, axis=1))
    moffs = np.cumsum([0] + [w for w in wins])
    mtot = int(mask_all.shape[1])

    # ---- build the Bass program (same for all cores) ----
    nc = bass.Bass()
    xs_h = nc.declare_dram_parameter("xs", [128, NC2, 2, BPC, FEATW], F8_DT,
                                     isOutput=False)
    wf_h = nc.declare_dram_parameter("wf", [128, NC2, 2, N], F8_DT,
                                     isOutput=False)
    mf_h = nc.declare_dram_parameter("mf", [128, mtot], F32, isOutput=False)
    z_h = nc.declare_dram_parameter("z", [128, BPC * nblk], F32, isOutput=True)

    ADD = mybir.AluOpType.add
    MULT = mybir.AluOpType.mult
    DR = mybir.MatmulPerfMode.DoubleRow

    with tile.TileContext(nc) as tc:
        with (
            tc.tile_pool(name="feat", bufs=1) as featp,
            tc.tile_pool(name="wpool", bufs=1) as wpool,
            tc.tile_pool(name="mpool", bufs=1) as mpool,
            tc.tile_pool(name="spool", bufs=4) as spool,
            tc.tile_pool(name="gpool", bufs=4) as gpool,
            tc.tile_pool(name="zpool", bufs=1) as zpool,
            tc.tile_pool(name="psum", bufs=1, space="PSUM") as psump,
        ):
            fts = [featp.tile([128, 2, BPC, FEATW], F8_DT, name=f"feat{c}")
                   for c in range(NC2)]
            mask_t = mpool.tile([128, mtot], F32)
            zAll = zpool.tile([128, BPC * nblk], F32)
            wgs = {}
            for g in range(ngrp):
                gcols = sblk[gbounds[g + 1]] - sblk[gbounds[g]]
                wgs[g] = wpool.tile([128, NC2, 2, gcols], F8_DT, name=f"wg{g}")

            def feat_half(c, h):
                return (fts[c][:, :, 2 * h:2 * h + 2, :],
                        xs_h[:, c, :, 2 * h:2 * h + 2, :])

            def wg_item(g):
                lo, hi = sblk[gbounds[g]], sblk[gbounds[g + 1]]
                return (wgs[g][:], wf_h[:, :, :, lo:hi])

            def mask_item(g):
                lo = int(moffs[gbounds[g]])
                hi = int(moffs[gbounds[g + 1]])
                return (mask_t[:, lo:hi], mf_h[:, lo:hi])

            # feat halves first (the whole PE stream gates on them), then W
            # groups + mask slices alternating queues, W0/W1 right after f0.
            sync_items = [[feat_half(0, 0)], [wg_item(0), mask_item(0)]]
            scal_items = [[feat_half(0, 1)], [wg_item(1), mask_item(1)]]
            for c in range(1, NC2):
                sync_items.append([feat_half(c, 0)])
                scal_items.append([feat_half(c, 1)])
            for g in range(2, ngrp):
                (sync_items if (g % 2 == 0) else scal_items).append(
                    [wg_item(g), mask_item(g)]
                )
            plan = {nc.sync: sync_items, nc.scalar: scal_items}
            maxlen = max(len(v) for v in plan.values())
            for k in range(maxlen):
                for eng, items in plan.items():
                    if k < len(items):
                        for dst, srcap in items[k]:
                            eng.dma_start(dst, srcap)

            for g in range(ngrp):
                blks = list(range(gbounds[g], gbounds[g + 1]))
                wg = wgs[g]
                glo = sblk[gbounds[g]]
                pms = {}
                for i in blks:
                    pms[i] = psump.tile([128, BPC, wins[i]], F32,
                                        name=f"pm{i}", tag=f"pm{i % 8}")
                for c in range(NC2):
                    for i in blks:
                        m, win, pfirst = ms[i], wins[i], pfirsts[i]
                        o = sblk[i] - glo
                        nc.tensor.matmul(
                            pms[i][0:m, :, :],
                            wg[:, c, :, o:o + m],
                            fts[c][:, :, :, pfirst:pfirst + win],
                            start=(c == 0),
                            stop=(c == NC2 - 1),
                            perf_mode=DR,
                        )
                # fused mask-mult + window-reduce, split DVE / GpSimd
                for i in blks:
                    m, win = ms[i], wins[i]
                    mo = int(moffs[i])
                    on_gp = False  # gpsimd cannot read PSUM directly
                    eng = nc.gpsimd if on_gp else nc.vector
                    pool = gpool if on_gp else spool
                    for bb in range(BPC):
                        sc = pool.tile([128, WINMAX], F32,
                                       tag=f"s{'g' if on_gp else 'v'}{bb}")
                        eng.scalar_tensor_tensor(
                            sc[0:m, 0:win],
                            pms[i][0:m, bb, :],
                            0.0,
                            mask_t[0:m, mo:mo + win],
                            ADD,
                            MULT,
                            accum_out=zAll[0:m, BPC * i + bb:BPC * i + bb + 1],
                        )
            nc.scalar.dma_start(z_h[:], zAll[:])

    _split_waits(nc)

    # ---- run on 8 cores: core id = bg*2 + dh ----
    xq = (x.reshape(B, D // 128, 128, P) * sx).astype(F8_NP)
    in_maps = []
    for core in range(NCORES):
        bg, dh = core // NDH, core % NDH
        xs_dev = np.zeros((128, NC2, 2, BPC, FEATW), F8_NP)
        # xq[b, chunk128, p, pos] -> [p, cc2, i, bb, pos]
        blkx = xq[BPC * bg:BPC * (bg + 1),
                  4 * dh:4 * (dh + 1)].reshape(BPC, NC2, 2, 128, P)
        xs_dev[:, :, :, :, :P] = blkx.transpose(3, 1, 2, 0, 4)
        in_maps.append({
            "xs": xs_dev,
            "wf": walls[dh],
            "mf": mask_all,
        })
    res = run_bass_kernel_spmd(nc, in_maps, core_ids=list(range(NCORES)))

    # ---- assemble: add D-halves, bias, elu(y)+1 ----
    y = np.empty((B, N), np.float32)
    for bg in range(NBG):
        z = res.results[NDH * bg]["z"] + res.results[NDH * bg + 1]["z"]
        for i, (s, e) in enumerate(blocks):
            idx = order[s:e]
            m = e - s
            y[BPC * bg:BPC * (bg + 1), idx] = z[0:m, BPC * i:BPC * (i + 1)].T
    y += b
    return np.where(y > 0, y + np.float32(1.0),
                    np.exp(np.minimum(y, np.float32(0.0)))).astype(np.float32)


# revision 11
# speedup vs baseline: 1.5511x; 1.0743x over previous
"""PoissonGaussianReadout forward on 8 trn2 NeuronCores.

Math (eval mode): each neuron n samples feat[b] (a [36,36,1024] image per
batch, 1024 = C*T channels) bilinearly at a fixed point mu[n], then takes a
per-neuron dot with W[n,:], adds b[n], applies elu(y)+1.

Strategy (v4):
  - Hybrid shard 4x2: 8 cores = 4 batch-groups (4 batches each) x 2 halves
    of the contraction dim D (512 channels each).  Cores emit LINEAR
    partial sums; the host adds the halves, bias, and elu on [16,4096].
  - fp8(e4m3) x and W with DoubleRow matmuls: x uses one global scale, W a
    per-neuron scale; both dequant factors fold into the (per-neuron) mask.
    Halves both the DMA stream and the PE time vs bf16.
  - Sort neurons by bilinear base cell p00; blocks of <=128 sorted neurons
    span a window of <=WINMAX flat positions.  Two DoubleRow matmuls per
    block (256-channel subtile pairs): psum[n,(b,j)] += Wblk^T @ feat-win.
  - Each neuron's 4 bilinear corners live at window offsets
    (p00-pfirst)+{0,1,36,37}; a host-built sparse mask [n, win] holds the
    bilinear weights (pre-divided by the fp8 scales).  The mask-multiply +
    window-reduce runs as scalar_tensor_tensor with accum, split between
    the DVE and GpSimd engines (blocks i%3==2 go to GpSimd).
  - DMA is need-ordered on the two HWDGE queues: feat chunk halves first,
    W block-groups + their mask slices just-in-time behind PE consumption.
"""
import sys
sys.path.insert(0, "/opt/trn_rl_repo")

import numpy as np

from concourse import bass, mybir, tile
from concourse.bass_utils import run_bass_kernel_spmd
import bass_rust

# problem constants
B, C, T, HH, WW = 16, 64, 16, 36, 36
N, D = 4096, C * T             # 4096 neurons, 1024 input dim
P = HH * WW                    # 1296 flat positions
NCORES = 8
NBG = 4                        # batch groups
NDH = 2                        # D halves
BPC = B // NBG                 # batches per core = 4
DH = D // NDH                  # channels per core = 512
NC2 = DH // 256                # 2 double-subtile (256-chan) passes per core
PAD = 38                       # max corner offset (37) + 1
WINMAX = 128                   # psum bank: BPC*WIN <= 512 fp32
FEATW = P + PAD                # padded feat width per (chunk, batch)
GRPN = 4                       # blocks per W DMA group

F32 = mybir.dt.float32

import ml_dtypes
F8_DT = mybir.dt.float8e4
F8_NP = ml_dtypes.float8_e4m3   # max normal 240
F8_CAP = np.float32(224.0)


def _split_waits(nc, max_waits=1):
    """Walrus in this image allows only ONE sem wait per instruction.
    Hoist extra waits onto injected same-engine NoOps placed immediately
    before the owning instruction (same engine + program order => same
    semantics)."""
    k = 0
    for fn in nc.m.functions:
        for blk in fn.blocks:
            insts = blk.instructions
            out = []
            for inst in insts:
                si = inst.sync_info
                if si is not None and si.on_wait and len(si.on_wait) > max_waits:
                    waits = list(si.on_wait)
                    for w in waits[:-max_waits]:
                        nop = mybir.InstNoOp(name=f"I-wsplit-{k}", ins=[], outs=[])
                        k += 1
                        nop.engine = inst.engine
                        nop.sync_info = bass_rust.SyncInfo(
                            on_wait=[w], on_update=[]
                        )
                        out.append(nop)
                    si.on_wait = waits[-max_waits:]
                    inst.sync_info = si
                out.append(inst)
            if len(out) != len(insts):
                insts.clear()
                insts.extend(out)


def _bilinear_tables(mu):
    """Per-neuron base cell p00, corner offsets (4) in {0,1,36,37}, corner
    weights (4), replicating reference float32 arithmetic exactly."""
    one, half = np.float32(1.0), np.float32(0.5)
    g = np.clip(mu.astype(np.float32), -one, one)
    ix = (g[:, 0] + one) * np.float32(WW * 0.5) - half
    iy = (g[:, 1] + one) * np.float32(HH * 0.5) - half
    x0 = np.floor(ix)
    y0 = np.floor(iy)
    wx1 = ix - x0
    wx0 = one - wx1
    wy1 = iy - y0
    wy0 = one - wy1

    xs = [x0, x0 + one]
    ys = [y0, y0 + one]
    wxs = [wx0, wx1]
    wys = [wy0, wy1]

    x0c = np.clip(x0, 0, WW - 1).astype(np.int64)
    y0c = np.clip(y0, 0, HH - 1).astype(np.int64)
    p00 = y0c * WW + x0c

    offs = np.zeros((4, N), np.int64)
    wgts = np.zeros((4, N), np.float32)
    k = 0
    for a in range(2):          # y corner
        for bb in range(2):     # x corner
            xx, yy = xs[bb], ys[a]
            valid = (xx >= 0) & (xx <= WW - 1) & (yy >= 0) & (yy <= HH - 1)
            xi = np.clip(xx, 0, WW - 1).astype(np.int64)
            yi = np.clip(yy, 0, HH - 1).astype(np.int64)
            offs[k] = yi * WW + xi - p00
            wgts[k] = (wys[a] * wxs[bb]) * valid.astype(np.float32)
            k += 1
    assert offs.min() >= 0 and offs.max() <= 37
    return p00, offs, wgts


def _make_blocks(p00_sorted):
    """Greedy blocks of <=128 sorted neurons with window <= WINMAX."""
    blocks = []  # (start, end) into sorted order
    s = 0
    n = len(p00_sorted)
    while s < n:
        pfirst = p00_sorted[s]
        e = s
        while e < n and e - s < 128 and (p00_sorted[e] - pfirst) + PAD <= WINMAX:
            e += 1
        blocks.append((s, e))
        s = e
    return blocks


def kernel(x, mu, sigma, W, b):
    x = np.ascontiguousarray(x, dtype=np.float32)
    W = np.ascontiguousarray(W, dtype=np.float32)
    b = np.asarray(b, dtype=np.float32)

    p00, offs, wgts = _bilinear_tables(mu)
    order = np.argsort(p00, kind="stable")
    p00s = p00[order]
    blocks = _make_blocks(p00s)
    nblk = len(blocks)

    # ---- fp8 quantization: global x scale, per-neuron W scale ----
    sx = F8_CAP / np.float32(max(np.abs(x).max(), 1e-30))
    sw = F8_CAP / np.maximum(np.abs(W).max(axis=1), 1e-30).astype(np.float32)
    Wq = (W * sw[:, None]).astype(F8_NP)    # [N, D]
    dequant = 1.0 / (sw * sx)               # [N] folded into the mask

    # per-block host data
    wins, pfirsts, ms, sblk = [], [], [], []
    mparts = []
    for i, (s, e) in enumerate(blocks):
        idx = order[s:e]
        m = e - s
        pfirst = int(p00s[s])
        win = int(p00s[e - 1]) - pfirst + PAD
        ms.append(m)
        pfirsts.append(pfirst)
        wins.append(win)
        sblk.append(s)
        # mask [128, win], fp8 dequant folded in
        mk = np.zeros((128, win), np.float32)
        rel = (p00[idx] - pfirst)  # [m]
        for k in range(4):
            np.add.at(mk[:m], (np.arange(m), rel + offs[k][idx]),
                      wgts[k][idx] * dequant[idx])
        mparts.append(mk)
    sblk.append(N)

    # W groups: first small so the PE can start early, the rest sized GRPN
    gbounds = [0, min(2, nblk)]
    while gbounds[-1] + GRPN < nblk:
        gbounds.append(gbounds[-1] + GRPN)
    if gbounds[-1] < nblk:
        gbounds.append(nblk)
    ngrp = len(gbounds) - 1

    # W packed per group with one contiguous row per partition:
    # group layout [128, NC2, 2, sum_m(group)]; groups concatenated flat.
    Ws = Wq[order]                          # [N, D] sorted
    gw_off = [0]
    for g in range(ngrp):
        gm = sblk[gbounds[g + 1]] - sblk[gbounds[g]]
        gw_off.append(gw_off[-1] + NC2 * 2 * gm)
    walls = []
    for dh in range(NDH):
        wl = (Ws[:, dh * DH:(dh + 1) * DH].T        # [512, N]
              .reshape(NC2, 2, 128, N).transpose(2, 0, 1, 3))  # [128,NC2,2,N]
        parts = []
        for g in range(ngrp):
            lo, hi = sblk[gbounds[g]], sblk[gbounds[g + 1]]
            parts.append(wl[:, :, :, lo:hi].reshape(128, -1))
        walls.append(np.ascontiguousarray(np.concatenate(parts, axis=1)))
    mask_all = np.ascontiguousarray(np.concatenate(mparts, axis=1))
    moffs = np.cumsum([0] + [w for w in wins])
    mtot = int(mask_all.shape[1])

    # ---- build the Bass program (same for all cores) ----
    nc = bass.Bass()
    xs_h = nc.declare_dram_parameter("xs", [128, NC2, 2, BPC, FEATW], F8_DT,
                                     isOutput=False)
    wf_h = nc.declare_dram_parameter("wf", [128, int(gw_off[-1])], F8_DT,
                                     isOutput=False)
    mf_h = nc.declare_dram_parameter("mf", [128, mtot], F32, isOutput=False)
    z_h = nc.declare_dram_parameter("z", [128, BPC * nblk], F32, isOutput=True)

    ADD = mybir.AluOpType.add
    MULT = mybir.AluOpType.mult
    DR = mybir.MatmulPerfMode.DoubleRow

    with tile.TileContext(nc) as tc:
        with (
            tc.tile_pool(name="feat", bufs=1) as featp,
            tc.tile_pool(name="wpool", bufs=1) as wpool,
            tc.tile_pool(name="mpool", bufs=1) as mpool,
            tc.tile_pool(name="spool", bufs=4) as spool,
            tc.tile_pool(name="gpool", bufs=4) as gpool,
            tc.tile_pool(name="zpool", bufs=1) as zpool,
            tc.tile_pool(name="psum", bufs=1, space="PSUM") as psump,
        ):
            fts = [featp.tile([128, 2, BPC, FEATW], F8_DT, name=f"feat{c}")
                   for c in range(NC2)]
            mask_t = mpool.tile([128, mtot], F32)
            zAll = zpool.tile([128, BPC * nblk], F32)
            wgs = {}
            for g in range(ngrp):
                gcols = sblk[gbounds[g + 1]] - sblk[gbounds[g]]
                wgs[g] = wpool.tile([128, NC2, 2, gcols], F8_DT, name=f"wg{g}")

            def feat_half(c, h):
                return (fts[c][:, :, 2 * h:2 * h + 2, :],
                        xs_h[:, c, :, 2 * h:2 * h + 2, :])

            def wg_item(g):
                return (wgs[g][:], wf_h[:, int(gw_off[g]):int(gw_off[g + 1])])

            def mask_item(g):
                lo = int(moffs[gbounds[g]])
                hi = int(moffs[gbounds[g + 1]])
                return (mask_t[:, lo:hi], mf_h[:, lo:hi])

            # feat halves first (the whole PE stream gates on them), then W
            # groups + mask slices alternating queues, W0/W1 right after f0.
            sync_items = [[feat_half(0, 0)], [wg_item(0), mask_item(0)]]
            scal_items = [[feat_half(0, 1)], [wg_item(1), mask_item(1)]]
            for c in range(1, NC2):
                sync_items.append([feat_half(c, 0)])
                scal_items.append([feat_half(c, 1)])
            for g in range(2, ngrp):
                (sync_items if (g % 2 == 0) else scal_items).append(
                    [wg_item(g), mask_item(g)]
                )
            plan = {nc.sync: sync_items, nc.scalar: scal_items}
            maxlen = max(len(v) for v in plan.values())
            for k in range(maxlen):
                for eng, items in plan.items():
                    if k < len(items):
                        for dst, srcap in items[k]:
                            eng.dma_start(dst, srcap)

            for g in range(ngrp):
                blks = list(range(gbounds[g], gbounds[g + 1]))
                wg = wgs[g]
                glo = sblk[gbounds[g]]
                pms = {}
                for i in blks:
                    pms[i] = psump.tile([128, BPC, wins[i]], F32,
                                        name=f"pm{i}", tag=f"pm{i % 8}")
                for c in range(NC2):
                    for i in blks:
                        m, win, pfirst = ms[i], wins[i], pfirsts[i]
                        o = sblk[i] - glo
                        nc.tensor.matmul(
                            pms[i][0:m, :, :],
                            wg[:, c, :, o:o + m],
                            fts[c][:, :, :, pfirst:pfirst + win],
                            start=(c == 0),
                            stop=(c == NC2 - 1),
                            perf_mode=DR,
                        )
                # fused mask-mult + window-reduce on DVE (the only engine
                # that can do tensor*tensor reads from PSUM)
                for i in blks:
                    m, win = ms[i], wins[i]
                    mo = int(moffs[i])
                    for bb in range(BPC):
                        sc = spool.tile([128, WINMAX], F32, tag=f"sv{bb}")
                        nc.vector.scalar_tensor_tensor(
                            sc[0:m, 0:win],
                            pms[i][0:m, bb, :],
                            0.0,
                            mask_t[0:m, mo:mo + win],
                            ADD,
                            MULT,
                            accum_out=zAll[0:m, BPC * i + bb:BPC * i + bb + 1],
                        )
            nc.scalar.dma_start(z_h[:], zAll[:])

    _split_waits(nc)

    # ---- run on 8 cores: core id = bg*2 + dh ----
    xq = (x.reshape(B, D // 128, 128, P) * sx).astype(F8_NP)
    in_maps = []
    for core in range(NCORES):
        bg, dh = core // NDH, core % NDH
        xs_dev = np.zeros((128, NC2, 2, BPC, FEATW), F8_NP)
        # xq[b, chunk128, p, pos] -> [p, cc2, i, bb, pos]
        blkx = xq[BPC * bg:BPC * (bg + 1),
                  4 * dh:4 * (dh + 1)].reshape(BPC, NC2, 2, 128, P)
        xs_dev[:, :, :, :, :P] = blkx.transpose(3, 1, 2, 0, 4)
        in_maps.append({
            "xs": xs_dev,
            "wf": walls[dh],
            "mf": mask_all,
        })
    res = run_bass_kernel_spmd(nc, in_maps, core_ids=list(range(NCORES)))

    # ---- assemble: add D-halves, bias, elu(y)+1 ----
    y = np.empty((B, N), np.float32)
    for bg in range(NBG):
        z = res.results[NDH * bg]["z"] + res.results[NDH * bg + 1]["z"]
        for i, (s, e) in enumerate(blocks):
            idx = order[s:e]
            m = e - s
            y[BPC * bg:BPC * (bg + 1), idx] = z[0:m, BPC * i:BPC * (i + 1)].T
    y += b
    return np.where(y > 0, y + np.float32(1.0),
                    np.exp(np.minimum(y, np.float32(0.0)))).astype(np.float32)


# revision 12
# speedup vs baseline: 1.6263x; 1.0485x over previous
"""PoissonGaussianReadout forward on 8 trn2 NeuronCores.

Math (eval mode): each neuron n samples feat[b] (a [36,36,1024] image per
batch, 1024 = C*T channels) bilinearly at a fixed point mu[n], then takes a
per-neuron dot with W[n,:], adds b[n], applies elu(y)+1.

Strategy (v4):
  - Hybrid shard 4x2: 8 cores = 4 batch-groups (4 batches each) x 2 halves
    of the contraction dim D (512 channels each).  Cores emit LINEAR
    partial sums; the host adds the halves, bias, and elu on [16,4096].
  - fp8(e4m3) x and W with DoubleRow matmuls: x uses one global scale, W a
    per-neuron scale; both dequant factors fold into the (per-neuron) mask.
    Halves both the DMA stream and the PE time vs bf16.
  - Sort neurons by bilinear base cell p00; blocks of <=128 sorted neurons
    span a window of <=WINMAX flat positions.  Two DoubleRow matmuls per
    block (256-channel subtile pairs): psum[n,(b,j)] += Wblk^T @ feat-win.
  - Each neuron's 4 bilinear corners live at window offsets
    (p00-pfirst)+{0,1,36,37}; a host-built sparse mask [n, win] holds the
    bilinear weights (pre-divided by the fp8 scales).  The mask-multiply +
    window-reduce runs as scalar_tensor_tensor with accum, split between
    the DVE and GpSimd engines (blocks i%3==2 go to GpSimd).
  - DMA is need-ordered on the two HWDGE queues: feat chunk halves first,
    W block-groups + their mask slices just-in-time behind PE consumption.
"""
import sys
sys.path.insert(0, "/opt/trn_rl_repo")

import numpy as np

from concourse import bass, mybir, tile
from concourse.bass_utils import run_bass_kernel_spmd
import bass_rust

# problem constants
B, C, T, HH, WW = 16, 64, 16, 36, 36
N, D = 4096, C * T             # 4096 neurons, 1024 input dim
P = HH * WW                    # 1296 flat positions
NCORES = 8
NBG = 4                        # batch groups
NDH = 2                        # D halves
BPC = B // NBG                 # batches per core = 4
DH = D // NDH                  # channels per core = 512
NC2 = DH // 256                # 2 double-subtile (256-chan) passes per core
PAD = 38                       # max corner offset (37) + 1
WINMAX = 128                   # psum bank: BPC*WIN <= 512 fp32
FEATW = P + PAD                # padded feat width per (chunk, batch)
GRPN = 4                       # blocks per W DMA group

F32 = mybir.dt.float32

import ml_dtypes
F8_DT = mybir.dt.float8e4
F8_NP = ml_dtypes.float8_e4m3   # max normal 240
F8_CAP = np.float32(224.0)


def _split_waits(nc, max_waits=1):
    """Walrus in this image allows only ONE sem wait per instruction.
    Hoist extra waits onto injected same-engine NoOps placed immediately
    before the owning instruction (same engine + program order => same
    semantics)."""
    k = 0
    for fn in nc.m.functions:
        for blk in fn.blocks:
            insts = blk.instructions
            out = []
            for inst in insts:
                si = inst.sync_info
                if si is not None and si.on_wait and len(si.on_wait) > max_waits:
                    waits = list(si.on_wait)
                    for w in waits[:-max_waits]:
                        nop = mybir.InstNoOp(name=f"I-wsplit-{k}", ins=[], outs=[])
                        k += 1
                        nop.engine = inst.engine
                        nop.sync_info = bass_rust.SyncInfo(
                            on_wait=[w], on_update=[]
                        )
                        out.append(nop)
                    si.on_wait = waits[-max_waits:]
                    inst.sync_info = si
                out.append(inst)
            if len(out) != len(insts):
                insts.clear()
                insts.extend(out)


def _bilinear_tables(mu):
    """Per-neuron base cell p00, corner offsets (4) in {0,1,36,37}, corner
    weights (4), replicating reference float32 arithmetic exactly."""
    one, half = np.float32(1.0), np.float32(0.5)
    g = np.clip(mu.astype(np.float32), -one, one)
    ix = (g[:, 0] + one) * np.float32(WW * 0.5) - half
    iy = (g[:, 1] + one) * np.float32(HH * 0.5) - half
    x0 = np.floor(ix)
    y0 = np.floor(iy)
    wx1 = ix - x0
    wx0 = one - wx1
    wy1 = iy - y0
    wy0 = one - wy1

    xs = [x0, x0 + one]
    ys = [y0, y0 + one]
    wxs = [wx0, wx1]
    wys = [wy0, wy1]

    x0c = np.clip(x0, 0, WW - 1).astype(np.int64)
    y0c = np.clip(y0, 0, HH - 1).astype(np.int64)
    p00 = y0c * WW + x0c

    offs = np.zeros((4, N), np.int64)
    wgts = np.zeros((4, N), np.float32)
    k = 0
    for a in range(2):          # y corner
        for bb in range(2):     # x corner
            xx, yy = xs[bb], ys[a]
            valid = (xx >= 0) & (xx <= WW - 1) & (yy >= 0) & (yy <= HH - 1)
            xi = np.clip(xx, 0, WW - 1).astype(np.int64)
            yi = np.clip(yy, 0, HH - 1).astype(np.int64)
            offs[k] = yi * WW + xi - p00
            wgts[k] = (wys[a] * wxs[bb]) * valid.astype(np.float32)
            k += 1
    assert offs.min() >= 0 and offs.max() <= 37
    return p00, offs, wgts


def _make_blocks(p00_sorted):
    """Greedy blocks of <=128 sorted neurons with window <= WINMAX."""
    blocks = []  # (start, end) into sorted order
    s = 0
    n = len(p00_sorted)
    while s < n:
        pfirst = p00_sorted[s]
        e = s
        while e < n and e - s < 128 and (p00_sorted[e] - pfirst) + PAD <= WINMAX:
            e += 1
        blocks.append((s, e))
        s = e
    return blocks


def kernel(x, mu, sigma, W, b):
    x = np.ascontiguousarray(x, dtype=np.float32)
    W = np.ascontiguousarray(W, dtype=np.float32)
    b = np.asarray(b, dtype=np.float32)

    p00, offs, wgts = _bilinear_tables(mu)
    order = np.argsort(p00, kind="stable")
    p00s = p00[order]
    blocks = _make_blocks(p00s)
    nblk = len(blocks)

    # ---- fp8 quantization: global x scale, per-neuron W scale ----
    sx = F8_CAP / np.float32(max(np.abs(x).max(), 1e-30))
    sw = F8_CAP / np.maximum(np.abs(W).max(axis=1), 1e-30).astype(np.float32)
    Wq = (W * sw[:, None]).astype(F8_NP)    # [N, D]
    dequant = 1.0 / (sw * sx)               # [N] folded into the mask

    # per-block host data
    wins, pfirsts, ms, sblk = [], [], [], []
    mparts = []
    for i, (s, e) in enumerate(blocks):
        idx = order[s:e]
        m = e - s
        pfirst = int(p00s[s])
        win = int(p00s[e - 1]) - pfirst + PAD
        ms.append(m)
        pfirsts.append(pfirst)
        wins.append(win)
        sblk.append(s)
        # mask [128, win], fp8 dequant folded in
        mk = np.zeros((128, win), np.float32)
        rel = (p00[idx] - pfirst)  # [m]
        for k in range(4):
            np.add.at(mk[:m], (np.arange(m), rel + offs[k][idx]),
                      wgts[k][idx] * dequant[idx])
        mparts.append(mk)
    sblk.append(N)

    # W groups: first small so the PE can start early, the rest sized GRPN
    gbounds = [0, min(2, nblk)]
    while gbounds[-1] + GRPN < nblk:
        gbounds.append(gbounds[-1] + GRPN)
    if gbounds[-1] < nblk:
        gbounds.append(nblk)
    ngrp = len(gbounds) - 1

    # W packed per group with one contiguous row per partition:
    # group layout [128, NC2, 2, sum_m(group)]; groups concatenated flat.
    Ws = Wq[order]                          # [N, D] sorted
    gw_off = [0]
    for g in range(ngrp):
        gm = sblk[gbounds[g + 1]] - sblk[gbounds[g]]
        gw_off.append(gw_off[-1] + NC2 * 2 * gm)
    walls = []
    for dh in range(NDH):
        wl = (Ws[:, dh * DH:(dh + 1) * DH].T        # [512, N]
              .reshape(NC2, 2, 128, N).transpose(2, 0, 1, 3))  # [128,NC2,2,N]
        parts = []
        for g in range(ngrp):
            lo, hi = sblk[gbounds[g]], sblk[gbounds[g + 1]]
            parts.append(wl[:, :, :, lo:hi].reshape(128, -1))
        walls.append(np.ascontiguousarray(np.concatenate(parts, axis=1)))
    mask_all = np.ascontiguousarray(np.concatenate(mparts, axis=1))
    moffs = np.cumsum([0] + [w for w in wins])
    mtot = int(mask_all.shape[1])

    # ---- build the Bass program (same for all cores) ----
    nc = bass.Bass()
    xs_h = nc.declare_dram_parameter("xs", [128, NC2, 2, BPC, FEATW], F8_DT,
                                     isOutput=False)
    wf_h = nc.declare_dram_parameter("wf", [128, int(gw_off[-1])], F8_DT,
                                     isOutput=False)
    mf_h = nc.declare_dram_parameter("mf", [128, mtot], F32, isOutput=False)
    z_h = nc.declare_dram_parameter("z", [128, BPC * nblk], F32, isOutput=True)

    ADD = mybir.AluOpType.add
    MULT = mybir.AluOpType.mult
    DR = mybir.MatmulPerfMode.DoubleRow

    with tile.TileContext(nc) as tc:
        with (
            tc.tile_pool(name="feat", bufs=1) as featp,
            tc.tile_pool(name="wpool", bufs=1) as wpool,
            tc.tile_pool(name="mpool", bufs=1) as mpool,
            tc.tile_pool(name="spool", bufs=4) as spool,
            tc.tile_pool(name="gpool", bufs=4) as gpool,
            tc.tile_pool(name="zpool", bufs=1) as zpool,
            tc.tile_pool(name="psum", bufs=1, space="PSUM") as psump,
        ):
            fts = [featp.tile([128, 2, BPC, FEATW], F8_DT, name=f"feat{c}")
                   for c in range(NC2)]
            mask_t = mpool.tile([128, mtot], F32)
            zAll = zpool.tile([128, BPC * nblk], F32)
            wgs = {}
            for g in range(ngrp):
                gcols = sblk[gbounds[g + 1]] - sblk[gbounds[g]]
                wgs[g] = wpool.tile([128, NC2, 2, gcols], F8_DT, name=f"wg{g}")

            def wg_item(g):
                return (wgs[g][:], wf_h[:, int(gw_off[g]):int(gw_off[g + 1])])

            def mask_item(g):
                lo = int(moffs[gbounds[g]])
                hi = int(moffs[gbounds[g + 1]])
                return (mask_t[:, lo:hi], mf_h[:, lo:hi])

            # One contiguous DMA per feat double-chunk (10.7KB rows, full
            # DMA rate), one per queue in parallel; then W groups + mask
            # slices alternating queues need-ordered.
            sync_items = [[(fts[0][:], xs_h[:, 0])]]
            scal_items = [[(fts[1][:], xs_h[:, 1])]]
            for g in range(ngrp):
                (sync_items if (g % 2 == 0) else scal_items).append(
                    [wg_item(g), mask_item(g)]
                )
            plan = {nc.sync: sync_items, nc.scalar: scal_items}
            maxlen = max(len(v) for v in plan.values())
            for k in range(maxlen):
                for eng, items in plan.items():
                    if k < len(items):
                        for dst, srcap in items[k]:
                            eng.dma_start(dst, srcap)

            for g in range(ngrp):
                blks = list(range(gbounds[g], gbounds[g + 1]))
                wg = wgs[g]
                glo = sblk[gbounds[g]]
                pms = {}
                for i in blks:
                    pms[i] = psump.tile([128, BPC, wins[i]], F32,
                                        name=f"pm{i}", tag=f"pm{i % 8}")
                for c in range(NC2):
                    for i in blks:
                        m, win, pfirst = ms[i], wins[i], pfirsts[i]
                        o = sblk[i] - glo
                        nc.tensor.matmul(
                            pms[i][0:m, :, :],
                            wg[:, c, :, o:o + m],
                            fts[c][:, :, :, pfirst:pfirst + win],
                            start=(c == 0),
                            stop=(c == NC2 - 1),
                            perf_mode=DR,
                        )
                # fused mask-mult + window-reduce on DVE (the only engine
                # that can do tensor*tensor reads from PSUM)
                for i in blks:
                    m, win = ms[i], wins[i]
                    mo = int(moffs[i])
                    for bb in range(BPC):
                        sc = spool.tile([128, WINMAX], F32, tag=f"sv{bb}")
                        nc.vector.scalar_tensor_tensor(
                            sc[0:m, 0:win],
                            pms[i][0:m, bb, :],
                            0.0,
                            mask_t[0:m, mo:mo + win],
                            ADD,
                            MULT,
                            accum_out=zAll[0:m, BPC * i + bb:BPC * i + bb + 1],
                        )
            nc.scalar.dma_start(z_h[:], zAll[:])

    _split_waits(nc)

    # ---- run on 8 cores: core id = bg*2 + dh ----
    xq = (x.reshape(B, D // 128, 128, P) * sx).astype(F8_NP)
    in_maps = []
    for core in range(NCORES):
        bg, dh = core // NDH, core % NDH
        xs_dev = np.zeros((128, NC2, 2, BPC, FEATW), F8_NP)
        # xq[b, chunk128, p, pos] -> [p, cc2, i, bb, pos]
        blkx = xq[BPC * bg:BPC * (bg + 1),
                  4 * dh:4 * (dh + 1)].reshape(BPC, NC2, 2, 128, P)
        xs_dev[:, :, :, :, :P] = blkx.transpose(3, 1, 2, 0, 4)
        in_maps.append({
            "xs": xs_dev,
            "wf": walls[dh],
            "mf": mask_all,
        })
    res = run_bass_kernel_spmd(nc, in_maps, core_ids=list(range(NCORES)))

    # ---- assemble: add D-halves, bias, elu(y)+1 ----
    y = np.empty((B, N), np.float32)
    for bg in range(NBG):
        z = res.results[NDH * bg]["z"] + res.results[NDH * bg + 1]["z"]
        for i, (s, e) in enumerate(blocks):
            idx = order[s:e]
            m = e - s
            y[BPC * bg:BPC * (bg + 1), idx] = z[0:m, BPC * i:BPC * (i + 1)].T
    y += b
    return np.where(y > 0, y + np.float32(1.0),
                    np.exp(np.minimum(y, np.float32(0.0)))).astype(np.float32)


# revision 21
# speedup vs baseline: 1.6822x; 1.0343x over previous
"""PoissonGaussianReadout forward on 8 trn2 NeuronCores.

Math (eval mode): each neuron n samples feat[b] (a [36,36,1024] image per
batch, 1024 = C*T channels) bilinearly at a fixed point mu[n], then takes a
per-neuron dot with W[n,:], adds b[n], applies elu(y)+1.

Strategy (v4):
  - Hybrid shard 4x2: 8 cores = 4 batch-groups (4 batches each) x 2 halves
    of the contraction dim D (512 channels each).  Cores emit LINEAR
    partial sums; the host adds the halves, bias, and elu on [16,4096].
  - fp8(e4m3) x and W with DoubleRow matmuls: x uses one global scale, W a
    per-neuron scale; both dequant factors fold into the (per-neuron) mask.
    Halves both the DMA stream and the PE time vs bf16.
  - Sort neurons by bilinear base cell p00; blocks of <=128 sorted neurons
    span a window of <=WINMAX flat positions.  Two DoubleRow matmuls per
    block (256-channel subtile pairs): psum[n,(b,j)] += Wblk^T @ feat-win.
  - Each neuron's 4 bilinear corners live at window offsets
    (p00-pfirst)+{0,1,36,37}; a host-built sparse mask [n, win] holds the
    bilinear weights (pre-divided by the fp8 scales).  The mask-multiply +
    window-reduce runs as scalar_tensor_tensor with accum, split between
    the DVE and GpSimd engines (blocks i%3==2 go to GpSimd).
  - DMA is need-ordered on the two HWDGE queues: feat chunk halves first,
    W block-groups + their mask slices just-in-time behind PE consumption.
"""
import sys
sys.path.insert(0, "/opt/trn_rl_repo")

import numpy as np

from concourse import bass, mybir, tile
from concourse.bass_utils import run_bass_kernel_spmd
import bass_rust

# problem constants
B, C, T, HH, WW = 16, 64, 16, 36, 36
N, D = 4096, C * T             # 4096 neurons, 1024 input dim
P = HH * WW                    # 1296 flat positions
NCORES = 8
NBG = 4                        # batch groups
NDH = 2                        # D halves
BPC = B // NBG                 # batches per core = 4
DH = D // NDH                  # channels per core = 512
NC2 = DH // 256                # 2 double-subtile (256-chan) passes per core
PAD = 38                       # max corner offset (37) + 1
WINMAX = 128                   # psum bank: BPC*WIN <= 512 fp32
FEATW = P + PAD                # padded feat width per (chunk, batch)
GRPN = 4                       # blocks per W DMA group

F32 = mybir.dt.float32

import ml_dtypes
F8_DT = mybir.dt.float8e4
F8_NP = ml_dtypes.float8_e4m3   # max normal 240
F8_CAP = np.float32(224.0)


def _split_waits(nc, max_waits=1):
    """Walrus in this image allows only ONE sem wait per instruction.
    Hoist extra waits onto injected same-engine NoOps placed immediately
    before the owning instruction (same engine + program order => same
    semantics)."""
    k = 0
    for fn in nc.m.functions:
        for blk in fn.blocks:
            insts = blk.instructions
            out = []
            for inst in insts:
                si = inst.sync_info
                if si is not None and si.on_wait and len(si.on_wait) > max_waits:
                    waits = list(si.on_wait)
                    for w in waits[:-max_waits]:
                        nop = mybir.InstNoOp(name=f"I-wsplit-{k}", ins=[], outs=[])
                        k += 1
                        nop.engine = inst.engine
                        nop.sync_info = bass_rust.SyncInfo(
                            on_wait=[w], on_update=[]
                        )
                        out.append(nop)
                    si.on_wait = waits[-max_waits:]
                    inst.sync_info = si
                out.append(inst)
            if len(out) != len(insts):
                insts.clear()
                insts.extend(out)


def _bilinear_tables(mu):
    """Per-neuron base cell p00, corner offsets (4) in {0,1,36,37}, corner
    weights (4), replicating reference float32 arithmetic exactly."""
    one, half = np.float32(1.0), np.float32(0.5)
    g = np.clip(mu.astype(np.float32), -one, one)
    ix = (g[:, 0] + one) * np.float32(WW * 0.5) - half
    iy = (g[:, 1] + one) * np.float32(HH * 0.5) - half
    x0 = np.floor(ix)
    y0 = np.floor(iy)
    wx1 = ix - x0
    wx0 = one - wx1
    wy1 = iy - y0
    wy0 = one - wy1

    xs = [x0, x0 + one]
    ys = [y0, y0 + one]
    wxs = [wx0, wx1]
    wys = [wy0, wy1]

    x0c = np.clip(x0, 0, WW - 1).astype(np.int64)
    y0c = np.clip(y0, 0, HH - 1).astype(np.int64)
    p00 = y0c * WW + x0c

    offs = np.zeros((4, N), np.int64)
    wgts = np.zeros((4, N), np.float32)
    k = 0
    for a in range(2):          # y corner
        for bb in range(2):     # x corner
            xx, yy = xs[bb], ys[a]
            valid = (xx >= 0) & (xx <= WW - 1) & (yy >= 0) & (yy <= HH - 1)
            xi = np.clip(xx, 0, WW - 1).astype(np.int64)
            yi = np.clip(yy, 0, HH - 1).astype(np.int64)
            offs[k] = yi * WW + xi - p00
            wgts[k] = (wys[a] * wxs[bb]) * valid.astype(np.float32)
            k += 1
    assert offs.min() >= 0 and offs.max() <= 37
    return p00, offs, wgts


def _make_blocks(p00_sorted):
    """Greedy blocks of <=128 sorted neurons with window <= WINMAX."""
    blocks = []  # (start, end) into sorted order
    s = 0
    n = len(p00_sorted)
    while s < n:
        pfirst = p00_sorted[s]
        e = s
        while e < n and e - s < 128 and (p00_sorted[e] - pfirst) + PAD <= WINMAX:
            e += 1
        blocks.append((s, e))
        s = e
    return blocks


def kernel(x, mu, sigma, W, b):
    x = np.ascontiguousarray(x, dtype=np.float32)
    W = np.ascontiguousarray(W, dtype=np.float32)
    b = np.asarray(b, dtype=np.float32)

    p00, offs, wgts = _bilinear_tables(mu)
    order = np.argsort(p00, kind="stable")
    p00s = p00[order]
    blocks = _make_blocks(p00s)
    nblk = len(blocks)

    # ---- fp8 quantization: global x scale, per-neuron W scale ----
    sx = F8_CAP / np.float32(max(np.abs(x).max(), 1e-30))
    sw = F8_CAP / np.maximum(np.abs(W).max(axis=1), 1e-30).astype(np.float32)
    Wq = (W * sw[:, None]).astype(F8_NP)    # [N, D]
    dequant = 1.0 / (sw * sx)               # [N] folded into the mask

    # per-block host data
    wins, pfirsts, ms, sblk = [], [], [], []
    mparts = []
    for i, (s, e) in enumerate(blocks):
        idx = order[s:e]
        m = e - s
        pfirst = int(p00s[s])
        win = int(p00s[e - 1]) - pfirst + PAD
        ms.append(m)
        pfirsts.append(pfirst)
        wins.append(win)
        sblk.append(s)
        # mask [128, win], fp8 dequant folded in
        mk = np.zeros((128, win), np.float32)
        rel = (p00[idx] - pfirst)  # [m]
        for k in range(4):
            np.add.at(mk[:m], (np.arange(m), rel + offs[k][idx]),
                      wgts[k][idx] * dequant[idx])
        mparts.append(mk)
    sblk.append(N)

    # W groups: first small so the PE can start early, the rest sized GRPN
    gbounds = [0, min(2, nblk)]
    while gbounds[-1] + GRPN < nblk:
        gbounds.append(gbounds[-1] + GRPN)
    if gbounds[-1] < nblk:
        gbounds.append(nblk)
    ngrp = len(gbounds) - 1

    # W packed per group with one contiguous row per partition:
    # group layout [128, NC2, 2, sum_m(group)]; groups concatenated flat.
    Ws = Wq[order]                          # [N, D] sorted
    gw_off = [0]
    for g in range(ngrp):
        gm = sblk[gbounds[g + 1]] - sblk[gbounds[g]]
        gw_off.append(gw_off[-1] + NC2 * 2 * gm)
    walls = []
    for dh in range(NDH):
        wl = (Ws[:, dh * DH:(dh + 1) * DH].T        # [512, N]
              .reshape(NC2, 2, 128, N).transpose(2, 0, 1, 3))  # [128,NC2,2,N]
        parts = []
        for g in range(ngrp):
            lo, hi = sblk[gbounds[g]], sblk[gbounds[g + 1]]
            parts.append(wl[:, :, :, lo:hi].reshape(128, -1))
        walls.append(np.ascontiguousarray(np.concatenate(parts, axis=1)))
    mask_all = np.ascontiguousarray(np.concatenate(mparts, axis=1))
    moffs = np.cumsum([0] + [w for w in wins])
    mtot = int(mask_all.shape[1])

    # ---- build the Bass program (same for all cores) ----
    nc = bass.Bass()
    xs_h = nc.declare_dram_parameter("xs", [128, NC2, 2, BPC, FEATW], F8_DT,
                                     isOutput=False)
    wf_h = nc.declare_dram_parameter("wf", [128, int(gw_off[-1])], F8_DT,
                                     isOutput=False)
    mf_h = nc.declare_dram_parameter("mf", [128, mtot], F32, isOutput=False)
    z_h = nc.declare_dram_parameter("z", [128, BPC * nblk], F32, isOutput=True)

    ADD = mybir.AluOpType.add
    MULT = mybir.AluOpType.mult
    DR = mybir.MatmulPerfMode.DoubleRow

    with tile.TileContext(nc) as tc:
        with (
            tc.tile_pool(name="feat", bufs=1) as featp,
            tc.tile_pool(name="wpool", bufs=1) as wpool,
            tc.tile_pool(name="mpool", bufs=1) as mpool,
            tc.tile_pool(name="spool", bufs=4) as spool,
            tc.tile_pool(name="gpool", bufs=4) as gpool,
            tc.tile_pool(name="zpool", bufs=1) as zpool,
            tc.tile_pool(name="psum", bufs=1, space="PSUM") as psump,
        ):
            fts = [featp.tile([128, 2, BPC, FEATW], F8_DT, name=f"feat{c}")
                   for c in range(NC2)]
            mask_t = mpool.tile([128, mtot], F32)
            zAll = zpool.tile([128, BPC * nblk], F32)
            wgs = {}
            for g in range(ngrp):
                gcols = sblk[gbounds[g + 1]] - sblk[gbounds[g]]
                wgs[g] = wpool.tile([128, NC2, 2, gcols], F8_DT, name=f"wg{g}")

            def wg_item(g):
                return (wgs[g][:], wf_h[:, int(gw_off[g]):int(gw_off[g + 1])])

            def mask_item(g):
                lo = int(moffs[gbounds[g]])
                hi = int(moffs[gbounds[g + 1]])
                return (mask_t[:, lo:hi], mf_h[:, lo:hi])

            # One contiguous DMA per feat double-chunk (10.7KB rows, full
            # DMA rate), one per queue in parallel; then W0+mask0 split
            # across both queues (earliest PE start), then the remaining
            # groups + mask slices alternating queues need-ordered.
            g0cols = sblk[gbounds[1]] - sblk[gbounds[0]]
            g0h = 2 * g0cols  # elements per cc2 slice of group 0
            sync_items = [[(fts[0][:], xs_h[:, 0])],
                          [(wgs[0][:, 0], wf_h[:, 0:g0h]), mask_item(0)]]
            scal_items = [[(fts[1][:], xs_h[:, 1])],
                          [(wgs[0][:, 1], wf_h[:, g0h:2 * g0h])]]
            for g in range(1, ngrp):
                (sync_items if (g % 2 == 0) else scal_items).append(
                    [wg_item(g), mask_item(g)]
                )
            plan = {nc.sync: sync_items, nc.scalar: scal_items}
            maxlen = max(len(v) for v in plan.values())
            for k in range(maxlen):
                for eng, items in plan.items():
                    if k < len(items):
                        for dst, srcap in items[k]:
                            eng.dma_start(dst, srcap)

            for g in range(ngrp):
                blks = list(range(gbounds[g], gbounds[g + 1]))
                wg = wgs[g]
                glo = sblk[gbounds[g]]
                pms = {}
                for i in blks:
                    pms[i] = psump.tile([128, BPC, wins[i]], F32,
                                        name=f"pm{i}", tag=f"pm{i % 8}")
                for c in range(NC2):
                    for i in blks:
                        m, win, pfirst = ms[i], wins[i], pfirsts[i]
                        o = sblk[i] - glo
                        nc.tensor.matmul(
                            pms[i][0:m, :, :],
                            wg[:, c, :, o:o + m],
                            fts[c][:, :, :, pfirst:pfirst + win],
                            start=(c == 0),
                            stop=(c == NC2 - 1),
                            perf_mode=DR,
                        )
                # fused mask-mult + window-reduce on DVE (the only engine
                # that can do tensor*tensor reads from PSUM)
                for i in blks:
                    m, win = ms[i], wins[i]
                    mo = int(moffs[i])
                    for bb in range(BPC):
                        sc = spool.tile([128, WINMAX], F32, tag=f"sv{bb}")
                        nc.vector.scalar_tensor_tensor(
                            sc[0:m, 0:win],
                            pms[i][0:m, bb, :],
                            0.0,
                            mask_t[0:m, mo:mo + win],
                            ADD,
                            MULT,
                            accum_out=zAll[0:m, BPC * i + bb:BPC * i + bb + 1],
                        )
                # store this group's partials right away (overlaps the rest)
                sl = slice(BPC * gbounds[g], BPC * gbounds[g + 1])
                seng = nc.sync if (g % 2 == 0) else nc.scalar
                seng.dma_start(z_h[:, sl], zAll[:, sl])

    _split_waits(nc)

    # ---- run on 8 cores: core id = bg*2 + dh ----
    xq = (x.reshape(B, D // 128, 128, P) * sx).astype(F8_NP)
    in_maps = []
    for core in range(NCORES):
        bg, dh = core // NDH, core % NDH
        xs_dev = np.zeros((128, NC2, 2, BPC, FEATW), F8_NP)
        # xq[b, chunk128, p, pos] -> [p, cc2, i, bb, pos]
        blkx = xq[BPC * bg:BPC * (bg + 1),
                  4 * dh:4 * (dh + 1)].reshape(BPC, NC2, 2, 128, P)
        xs_dev[:, :, :, :, :P] = blkx.transpose(3, 1, 2, 0, 4)
        in_maps.append({
            "xs": xs_dev,
            "wf": walls[dh],
            "mf": mask_all,
        })
    res = run_bass_kernel_spmd(nc, in_maps, core_ids=list(range(NCORES)))

    # ---- assemble: add D-halves, bias, elu(y)+1 ----
    y = np.empty((B, N), np.float32)
    for bg in range(NBG):
        z = res.results[NDH * bg]["z"] + res.results[NDH * bg + 1]["z"]
        for i, (s, e) in enumerate(blocks):
            idx = order[s:e]
            m = e - s
            y[BPC * bg:BPC * (bg + 1), idx] = z[0:m, BPC * i:BPC * (i + 1)].T
    y += b
    return np.where(y > 0, y + np.float32(1.0),
                    np.exp(np.minimum(y, np.float32(0.0)))).astype(np.float32)
